# revision 24
# baseline (speedup 1.0000x reference)
"""Trainium2 Bass kernel for nn_Attn_86784109183632.

Transformer block: LN1 -> +sinusoidal PE -> linear (efficient) attention ->
w_out + residual -> LN2 -> 3-layer gelu MLP + residual.
B=4, S=4096, D=1024, H=16, dh=64.

Sharding: data-parallel over (batch, seq-half) -> 8 cores x 2048 tokens.
The only cross-core term is the k-softmax normalizer and k^T v context
(sums over the sequence axis), reduced with a tiny pairwise AllReduce
([128,1024] fp32) between the two cores holding the same batch, overlapped
with the q projection + q softmax.

Precision plan (tolerance is rel-max 2e-2; measured ~1.3e-2 in numpy sim):
  - x / residuals / LN statistics: fp32(r)
  - attention path (h, w_qkv, exp(k), v, softmaxed q, ctx): bf16
  - MLP + w_out GEMMs: fp8 e4m3 with DoubleRow perf mode (2 k-blocks per
    matmul pass), weights SBUF-resident, activations quantized on the fly
    by the Activation engine into paired [128, 2, 512] tiles.
Softmaxed q stays SBUF-resident between the q phase and phase B (no DRAM
spill). All activations are dim-major [dims, tokens].
"""

import sys

if "/opt/trn_rl_repo" not in sys.path:
    sys.path.insert(0, "/opt/trn_rl_repo")

import ml_dtypes
import numpy as np

import concourse.mybir as mybir
import concourse.tile as tile
from concourse import bacc
from concourse.alu_op_type import AluOpType
from concourse.bass_utils import run_bass_kernel_spmd

P = 128
D = 1024
DD = 2048  # mlp hidden
H = 16
DH = 64
B = 4
S_FULL = 4096
NCORES = 8
EPS = 1e-6

FR = mybir.dt.float32r
F32 = mybir.dt.float32
BF = mybir.dt.bfloat16
F8 = mybir.dt.float8e4
AF = mybir.ActivationFunctionType
DRM = mybir.MatmulPerfMode.DoubleRow

DT = D // P        # 8 d-tiles
DDT = DD // P      # 16 mlp-tiles
NCH = 512          # token chunk (one fp32 psum bank)


def _ctx_col(i):
    """Free-dim offset of head-pair block i inside ctx psum (4 pairs/bank)."""
    return 512 * (i // 4) + 65 * (i % 4)


def build_graph(T):
    """Build the SPMD graph for T tokens per core. T % 512 == 0."""
    assert T % NCH == 0
    TT = T // P           # token tiles
    NC = T // NCH         # token chunks

    nc = bacc.Bacc("TRN2", target_bir_lowering=False, debug=False,
                   num_devices=NCORES)

    tn = {}
    tn["xT"] = nc.dram_tensor("xT", [D, T], FR, kind="ExternalInput")
    tn["peb"] = nc.dram_tensor("peb", [D, T], BF, kind="ExternalInput")
    # q/kv weights pair-packed for DoubleRow: row 128t+p, col j*M+c holds
    # w[(2t+j)*128+p, c]
    tn["wq"] = nc.dram_tensor("wq", [D // 2, 2 * D], F8, kind="ExternalInput")
    tn["wkv"] = nc.dram_tensor("wkv", [D // 2, 4 * D], F8, kind="ExternalInput")
    tn["wout"] = nc.dram_tensor("wout", [D, D], F8, kind="ExternalInput")  # col-block
    tn["w1"] = nc.dram_tensor("w1", [DD, D], F8, kind="ExternalInput")  # col-block
    tn["w2"] = nc.dram_tensor("w2", [DD, DD], F8, kind="ExternalInput")  # col-block
    tn["w3"] = nc.dram_tensor("w3", [D, DD], F8, kind="ExternalInput")  # col-block
    # per-dim vectors laid out [128, n_tiles] (column t = dims 128t..128t+127)
    for name, nt in [("lng1", DT), ("lng2", DT), ("lnb2", DT),
                     ("bout", DT), ("b1", DDT), ("b2", DDT), ("b3", DT)]:
        tn[name] = nc.dram_tensor(name, [P, nt], F32, kind="ExternalInput")
    tn["ones"] = nc.dram_tensor("ones", [P, P], FR, kind="ExternalInput")
    tn["indsum"] = nc.dram_tensor("indsum", [DT * P, 32], BF, kind="ExternalInput")
    tn["indbc"] = nc.dram_tensor("indbc", [DT * 32, P], BF, kind="ExternalInput")
    tn["out"] = nc.dram_tensor("out", [D, T], F32, kind="ExternalOutput")

    PHASE_MARKS.clear()
    with tile.TileContext(nc) as tc:
        _build_body(nc, tc, T, TT, NC, tn)
    import json as _json
    _json.dump(PHASE_MARKS, open("/tmp/phase_marks.json", "w"))
    nc.compile()
    return nc


PHASE_MARKS = []


def _mark(nc, label):
    PHASE_MARKS.append((label, nc.next_id()))


def _build_body(nc, tc, T, TT, NC, tn):
    xT, out_d = tn["xT"], tn["out"]
    RG = [[0, 1], [2, 3], [4, 5], [6, 7]]

    with tc.tile_pool(name="const", bufs=1) as const, \
         tc.tile_pool(name="ctx_hold", bufs=1) as ctx_hold, \
         tc.tile_pool(name="qs_hold", bufs=1) as qs_hold, \
         tc.tile_pool(name="dram", bufs=1, space="DRAM") as dram:

        # ------------- constants -------------
        ones_t = const.tile([P, P], FR)
        nc.sync.dma_start(ones_t[:], tn["ones"][:])
        indsum_t, indbc_t = [], []
        for t in range(DT):
            it = const.tile([P, 32], BF, tag=f"indsum{t}", name=f"indsum{t}")
            nc.sync.dma_start(it[:], tn["indsum"][t * P:(t + 1) * P, :])
            indsum_t.append(it)
            bt = const.tile([32, P], BF, tag=f"indbc{t}", name=f"indbc{t}")
            nc.sync.dma_start(bt[:], tn["indbc"][t * 32:(t + 1) * 32, :])
            indbc_t.append(bt)
        vecs = {}
        for name, nt in [("lng1", DT), ("lng2", DT), ("lnb2", DT),
                         ("bout", DT), ("b1", DDT), ("b2", DDT), ("b3", DT)]:
            v = const.tile([P, nt], F32, tag=name)
            nc.sync.dma_start(v[:], tn[name][:])
            vecs[name] = v

        eps_t = const.tile([P, 1], F32, tag="eps", name="eps")
        nc.vector.memset(eps_t[:], EPS)
        ctxg_sb = ctx_hold.tile([P, 1024], F32)   # ctx after AllReduce

        # softmaxed q, SBUF-resident through phase B
        qs_sb = [qs_hold.tile([P, T], BF, tag=f"qs{m}", name=f"qs{m}")
                 for m in range(DT)]

        ar_in = dram.tile([P, 1024], F32, tag="ar_in", name="ar_in")
        ar_out = dram.tile([P, 1024], F32, tag="ar_out", name="ar_out")

        def ln_stats_to_scales(mu_ps, ms_ps, pool, tagsfx, tmp_pool=None):
            """mu_ps/ms_ps: psum [P, NCH] broadcast sums of x and x^2 over D.
            Returns (rstd_b, murstd_b) SBUF [P, NCH] f32."""
            tpool = tmp_pool if tmp_pool is not None else pool
            mu_n = tpool.tile([P, NCH], F32, tag="t_mun" + tagsfx)
            nc.scalar.mul(mu_n[:], mu_ps[:], 1.0 / D)
            var = tpool.tile([P, NCH], F32, tag="t_var" + tagsfx)
            nc.scalar.activation(var[:], mu_ps[:], AF.Square, scale=1.0 / D)
            ex2 = tpool.tile([P, NCH], F32, tag="t_ex2" + tagsfx)
            nc.scalar.mul(ex2[:], ms_ps[:], 1.0 / D)
            nc.vector.tensor_sub(var[:], ex2[:], var[:])
            sd = tpool.tile([P, NCH], F32, tag="t_sd" + tagsfx)
            nc.scalar.activation(sd[:], var[:], AF.Sqrt, bias=eps_t[:])
            rb = pool.tile([P, NCH], F32, tag="rb" + tagsfx)
            nc.vector.reciprocal(rb[:], sd[:])
            mb = pool.tile([P, NCH], F32, tag="mb" + tagsfx)
            nc.vector.tensor_mul(mb[:], mu_n[:], rb[:])
            return rb, mb

        # =================================================================
        # PHASE A
        # =================================================================
        with tc.tile_pool(name="h_pool", bufs=1) as h_pool, \
             tc.tile_pool(name="expq_pool", bufs=1) as eqp:
            # h as fp8 k-pair tiles for DoubleRow: h_p[t][:, j, s] = h block 2t+j
            h_p = [h_pool.tile([P, 2, T], F8, tag=f"h{t}", name=f"h{t}")
                   for t in range(DT // 2)]
            expq_a = [[None] * DT for _ in range(NC)]

            # ---- fused: LN1 + kv-GEMM + ctx + q-GEMM, chunk-pipelined ----
            with tc.tile_pool(name="wkv_pool", bufs=1) as wkv_pool, \
                 tc.tile_pool(name="ln1_work", bufs=2) as lnw, \
                 tc.tile_pool(name="ln1_tmp", bufs=1) as lntmp, \
                 tc.tile_pool(name="ln1_x", bufs=2) as lnx, \
                 tc.tile_pool(name="ln1_stream", bufs=2) as lns, \
                 tc.tile_pool(name="kv_work", bufs=2) as kvw, \
                 tc.tile_pool(name="kv_ev", bufs=1) as kvev, \
                 tc.tile_pool(name="ln1_psum", bufs=1, space="PSUM") as lnp, \
                 tc.tile_pool(name="mm_psum", bufs=2, space="PSUM") as kvp_pool, \
                 tc.tile_pool(name="ctx_psum", bufs=1, space="PSUM") as ctxp_pool:
                qp_pool = kvp_pool
                ctx_ps = ctxp_pool.tile([P, 1024], F32, tag="ctx", name="ctx")
                wkv_t, wq_t = [], []

                def load_w():
                    for t in range(DT // 2):
                        wt = wkv_pool.tile([P, 2, 2 * D], F8, tag=f"wkv{t}",
                                           name=f"wkv{t}")
                        nc.sync.dma_start(
                            wt[:].rearrange("p j c -> p (j c)"),
                            tn["wkv"][t * P:(t + 1) * P, :])
                        wkv_t.append(wt)
                        qt = wkv_pool.tile([P, 2, D], F8, tag=f"wq{t}",
                                           name=f"wq{t}")
                        nc.sync.dma_start(
                            qt[:].rearrange("p j c -> p (j c)"),
                            tn["wq"][t * P:(t + 1) * P, :])
                        wq_t.append(qt)

                stats_ps = {}
                xcur = {}

                def ln1_stats(c):
                    cs = slice(c * NCH, (c + 1) * NCH)
                    mu = lnp.tile([P, NCH], F32, tag=f"mu{c % 2}", name=f"mu{c % 2}")
                    ms = lnp.tile([P, NCH], F32, tag=f"ms{c % 2}", name=f"ms{c % 2}")
                    xs = []
                    for k in range(DT):
                        xk = lnx.tile([P, NCH], FR, tag=f"xc{k}", name=f"xc{k}")
                        nc.sync.dma_start(xk[:], xT[k * P:(k + 1) * P, cs])
                        sq = lns.tile([P, NCH], FR, tag="sq", name="sq")
                        nc.scalar.activation(sq[:], xk[:], AF.Square)
                        nc.tensor.matmul(mu[:], ones_t[:], xk[:],
                                         start=(k == 0), stop=(k == DT - 1))
                        nc.tensor.matmul(ms[:], ones_t[:], sq[:],
                                         start=(k == 0), stop=(k == DT - 1))
                        xs.append(xk)
                    stats_ps[c] = (mu, ms)
                    xcur[c] = xs

                def ln1_apply(c):
                    cs = slice(c * NCH, (c + 1) * NCH)
                    mu, ms = stats_ps.pop(c)
                    rb, mb = ln_stats_to_scales(mu, ms, lnw, "1", lntmp)
                    xs = xcur.pop(c)
                    for k in range(DT):
                        pk = lns.tile([P, NCH], BF, tag="pe", name="pe")
                        nc.sync.dma_start(pk[:], tn["peb"][k * P:(k + 1) * P, cs])
                        tmp = lns.tile([P, NCH], F32, tag="lnt", name="lnt")
                        nc.vector.tensor_mul(tmp[:], xs[k][:], rb[:])
                        nc.vector.tensor_sub(tmp[:], tmp[:], mb[:])
                        # h = (LN1(x) + (pe + b)/g) * g — gain applied via the
                        # ACT copy's scale slot; host sends peb = (pe + b)/g
                        nc.vector.tensor_add(tmp[:], tmp[:], pk[:])
                        with nc.allow_low_precision(reason="h fp8 for DR gemms"):
                            nc.scalar.activation(h_p[k // 2][:, k % 2, cs],
                                                 tmp[:], AF.Identity,
                                                 scale=vecs["lng1"][:, k:k + 1])

                pending = []  # (ek, vv, global_tt) awaiting ctx matmuls

                def flush_ctx(last=False):
                    while pending:
                        ek, vv, pt = pending.pop(0)
                        for h16 in range(H):
                            i, j = h16 // 2, h16 % 2
                            c0 = _ctx_col(i)
                            nc.tensor.matmul(
                                ctx_ps[64 * j:64 * j + 64, c0:c0 + 65],
                                ek[:, 64 * h16:64 * h16 + 64],
                                vv[:, h16 * 65:(h16 + 1) * 65],
                                start=(pt == 0 and h16 in (0, 1, 8, 9)),
                                stop=(pt == TT - 1 and h16 in (6, 7, 14, 15)))

                def kv_ctx(c):
                    for lt in range(NCH // P):
                        tt = c * (NCH // P) + lt
                        ts_ = slice(tt * P, (tt + 1) * P)
                        ek = kvw.tile([P, D], BF, tag="ek", name="ek")
                        vv = kvw.tile([P, H * 65], BF, tag="vv", name="vv")
                        vv3 = vv[:].rearrange("p (h e) -> p h e", e=65)
                        for n in range(4):
                            pn = kvp_pool.tile([P, 512], F32, tag="kv", name="kv")
                            for t in range(DT // 2):
                                nc.tensor.matmul(
                                    pn[:], h_p[t][:, :, ts_],
                                    wkv_t[t][:, :, n * 512:(n + 1) * 512],
                                    start=(t == 0), stop=(t == DT // 2 - 1),
                                    perf_mode=DRM)
                            if n < 2:
                                nc.scalar.activation(ek[:, n * 512:(n + 1) * 512],
                                                     pn[:], AF.Exp)
                            else:
                                nc.vector.tensor_copy(
                                    vv3[:, (n - 2) * 8:(n - 1) * 8, 0:64],
                                    pn[:].rearrange("p (h e) -> p h e", e=64))
                        nc.vector.memset(vv3[:, :, 64:65], 1.0)
                        flush_ctx()
                        pending.append((ek, vv, tt))

                def qexp(c):
                    cs = slice(c * NCH, (c + 1) * NCH)
                    for m in range(DT):
                        qp = qp_pool.tile([P, NCH], F32, tag="kv", name="q")
                        for t in range(DT // 2):
                            nc.tensor.matmul(
                                qp[:], wq_t[t][:, :, m * P:(m + 1) * P],
                                h_p[t][:, :, cs],
                                start=(t == 0), stop=(t == DT // 2 - 1),
                                perf_mode=DRM)
                        eq = eqp.tile([P, NCH], BF, tag=f"expq{c}_{m}",
                                      name=f"expq{c}_{m}")
                        nc.scalar.activation(eq[:], qp[:], AF.Exp)
                        expq_a[c][m] = eq

                _mark(nc, 'A:start')
                load_w()
                ln1_stats(0)
                for c in range(NC):
                    if c + 1 < NC:
                        ln1_stats(c + 1)
                    ln1_apply(c)
                    kv_ctx(c)
                flush_ctx(last=True)

                _mark(nc, 'A:ctx_evict')
                ctx_sb = kvev.tile([P, 1024], F32, tag="ctxev", name="ctxev")
                nc.vector.tensor_copy(ctx_sb[:], ctx_ps[:])
                nc.sync.dma_start(ar_in[:], ctx_sb[:])

                # q-GEMM after the evict in program order: the scheduler
                # pulls it forward into main-loop bubbles, and whatever is
                # left covers the AllReduce latency window.
                for c in range(NC):
                    qexp(c)

            nc.gpsimd.collective_compute(
                "AllReduce", AluOpType.add, replica_groups=RG,
                ins=[ar_in[:].opt()], outs=[ar_out[:].opt()])
            nc.sync.dma_start(ctxg_sb[:], ar_out[:])

            _mark(nc, 'A:q')
            # ---------- softmax tail: 1/sum + broadcast, overlaps AllReduce
            with tc.tile_pool(name="q_small", bufs=3) as qsm, \
                 tc.tile_pool(name="bc_psum", bufs=2, space="PSUM") as bc_pool, \
                 tc.tile_pool(name="ssum_psum", bufs=2, space="PSUM") as sp_pool:
                for c in range(NC):
                    cs = slice(c * NCH, (c + 1) * NCH)
                    expq = expq_a[c]
                    s_ps = sp_pool.tile([32, NCH], F32, tag="ssum", name="ssum")
                    for m in range(DT):
                        nc.tensor.matmul(s_ps[:], indsum_t[m][:], expq[m][:],
                                         start=(m == 0), stop=(m == DT - 1))
                    rs = qsm.tile([32, NCH], BF, tag="recS", name="recS")
                    nc.vector.tensor_copy(rs[:], s_ps[:])
                    with nc.allow_low_precision(reason="softmax scale in bf16"):
                        nc.vector.reciprocal(rs[0:H, :], s_ps[0:H, :])
                    for m in range(DT):
                        bc = bc_pool.tile([P, NCH], F32, tag="bc", name="bc")
                        nc.tensor.matmul(bc[:], indbc_t[m][:], rs[:],
                                         start=True, stop=True)
                        with nc.allow_low_precision(reason="softmaxed q in bf16"):
                            nc.vector.tensor_mul(qs_sb[m][:, cs], expq[m][:], bc[:])

        # phase-B weights, fp8, SBUF-resident (loaded once at B start; the
        # first consumers — wout of chunk 0 — need only the first 1 MB).
        bw_cm = tc.tile_pool(name="bw_hold", bufs=1)
        bw = bw_cm.__enter__()
        wout_t = [bw.tile([P, D], F8, tag=f"wo{m}", name=f"wo{m}")
                  for m in range(DT)]
        w1_t = [bw.tile([P, D], F8, tag=f"w1_{m}", name=f"w1_{m}")
                for m in range(DDT)]
        w2_t = [bw.tile([P, DD], F8, tag=f"w2_{m}", name=f"w2_{m}")
                for m in range(DDT)]
        w3_t = [bw.tile([P, DD], F8, tag=f"w3_{m}", name=f"w3_{m}")
                for m in range(DT)]
        for m in range(DT):
            nc.sync.dma_start(wout_t[m][:], tn["wout"][m * P:(m + 1) * P, :])
        for m in range(DDT):
            nc.sync.dma_start(w1_t[m][:], tn["w1"][m * P:(m + 1) * P, :])
        for m in range(DDT):
            nc.sync.dma_start(w2_t[m][:], tn["w2"][m * P:(m + 1) * P, :])
        for m in range(DT):
            nc.sync.dma_start(w3_t[m][:], tn["w3"][m * P:(m + 1) * P, :])

        _mark(nc, 'ctxnorm')
        # normalize ctx into block-diagonal head-pair lhsT tiles (bf16):
        # ctxd[:, 128i:128(i+1)] = [[ctx_{2i}*zr, 0], [0, ctx_{2i+1}*zr]]
        bhold_cm = tc.tile_pool(name="b_hold", bufs=1)
        bhold = bhold_cm.__enter__()
        ctxd_sb = bhold.tile([P, 1024], BF, tag="ctxd", name="ctxd")
        zr_sb = bhold.tile([P, 8], F32, tag="zr", name="zr")
        for i in range(8):
            c0 = _ctx_col(i)
            nc.vector.reciprocal(zr_sb[:, i:i + 1], ctxg_sb[:, c0 + 64:c0 + 65])
        nc.scalar.mul(zr_sb[:], zr_sb[:], DH ** -0.5)
        nc.vector.memset(ctxd_sb[:], 0.0)
        for h16 in range(H):
            i, j = h16 // 2, h16 % 2
            c0 = _ctx_col(i)
            nc.vector.tensor_scalar(
                ctxd_sb[64 * j:64 * j + 64, 128 * i + 64 * j:128 * i + 64 * j + 64],
                ctxg_sb[64 * j:64 * j + 64, c0:c0 + 64],
                zr_sb[64 * j:64 * j + 64, i:i + 1], None, AluOpType.mult)

        # =================================================================
        # PHASE B: per token chunk attn -> w_out+res -> LN2 -> MLP+res
        # (cross-chunk pipelined; MLP + w_out GEMMs in fp8 DoubleRow)
        # =================================================================
        with tc.tile_pool(name="b_attp", bufs=1) as bap, \
             tc.tile_pool(name="b_act2", bufs=2) as bact2, \
             tc.tile_pool(name="b_stream", bufs=4) as bstr, \
             tc.tile_pool(name="b_y", bufs=1) as by_pool, \
             tc.tile_pool(name="b_work", bufs=2) as bw2, \
             tc.tile_pool(name="b_lnw", bufs=1) as blnw, \
             tc.tile_pool(name="b_psum", bufs=2, space="PSUM") as bp, \
             tc.tile_pool(name="b_stat_psum", bufs=1, space="PSUM") as bsp:
            x2_c = {}
            h2_c = {}
            stats_c = {}

            def stage_a(n):
                cs = slice(n * NCH, (n + 1) * NCH)
                _mark(nc, f'B{n}:attn')
                att_p = [bap.tile([P, 2, NCH], F8, tag=f"attp{t}",
                                  name=f"attp{t}") for t in range(DT // 2)]
                for i in range(DT):
                    ap_ps = bp.tile([P, NCH], F32, tag="attn", name="attn")
                    nc.tensor.matmul(ap_ps[:], ctxd_sb[:, P * i:P * (i + 1)],
                                     qs_sb[i][:, cs], start=True, stop=True)
                    with nc.allow_low_precision(reason="attn out fp8 for DR gemm"):
                        nc.scalar.mul(att_p[i // 2][:, i % 2, :], ap_ps[:], 1.0)
                _mark(nc, f'B{n}:wout')
                x2_t = []
                mu2 = bsp.tile([P, NCH], F32, tag="mu2", name="mu2")
                ms2 = bsp.tile([P, NCH], F32, tag="ms2", name="ms2")
                for m in range(DT):
                    wo_ps = bp.tile([P, NCH], F32, tag="wout", name="wout")
                    w3d = wout_t[m][:].rearrange("p (k c) -> p k c", c=P)
                    for t in range(DT // 2):
                        nc.tensor.matmul(wo_ps[:], w3d[:, 2 * t:2 * t + 2, :],
                                         att_p[t][:, :, :],
                                         start=(t == 0), stop=(t == DT // 2 - 1),
                                         perf_mode=DRM)
                    xc = bw2.tile([P, NCH], FR, tag="xc", name="xc")
                    nc.sync.dma_start(xc[:], xT[m * P:(m + 1) * P, cs])
                    x2 = bact2.tile([P, NCH], FR, tag=f"x2_{m}", name=f"x2_{m}")
                    nc.vector.scalar_tensor_tensor(
                        x2[:], wo_ps[:], vecs["bout"][:, m:m + 1], xc[:],
                        AluOpType.add, AluOpType.add)
                    x2_t.append(x2)
                    sq = bw2.tile([P, NCH], FR, tag="sq2", name="sq2")
                    nc.scalar.activation(sq[:], x2[:], AF.Square)
                    nc.tensor.matmul(mu2[:], ones_t[:], x2[:],
                                     start=(m == 0), stop=(m == DT - 1))
                    nc.tensor.matmul(ms2[:], ones_t[:], sq[:],
                                     start=(m == 0), stop=(m == DT - 1))
                x2_c[n] = x2_t
                stats_c[n] = (mu2, ms2)

            def stage_ln(n):
                _mark(nc, f'B{n}:ln2')
                mu2, ms2 = stats_c.pop(n)
                rstd, murstd = ln_stats_to_scales(mu2, ms2, blnw, "2")
                h2p = [bact2.tile([P, 2, NCH], F8, tag=f"h2p{t}",
                                  name=f"h2p{t}") for t in range(DT // 2)]
                for m in range(DT):
                    tmp = bw2.tile([P, NCH], F32, tag="h2t", name="h2t")
                    nc.vector.tensor_mul(tmp[:], x2_c[n][m][:], rstd[:])
                    nc.vector.tensor_sub(tmp[:], tmp[:], murstd[:])
                    # h2 = tmp*g2 + b2ln, quantized to fp8 on the ACT engine
                    with nc.allow_low_precision(reason="h2 fp8 for DR gemm"):
                        nc.scalar.activation(
                            h2p[m // 2][:, m % 2, :], tmp[:], AF.Identity,
                            scale=vecs["lng2"][:, m:m + 1],
                            bias=vecs["lnb2"][:, m:m + 1])
                h2_c[n] = h2p

            def stage_mlp(n):
                cs = slice(n * NCH, (n + 1) * NCH)
                h2p = h2_c.pop(n)
                x2_t = x2_c.pop(n)
                _mark(nc, f'B{n}:y1')
                y1p = [by_pool.tile([P, 2, NCH], F8, tag=f"y1p{t}",
                                    name=f"y1p{t}") for t in range(DDT // 2)]
                for m in range(DDT):
                    y_ps = bp.tile([P, NCH], F32, tag="mlp", name="mlp")
                    w3d = w1_t[m][:].rearrange("p (k c) -> p k c", c=P)
                    for t in range(DT // 2):
                        nc.tensor.matmul(y_ps[:], w3d[:, 2 * t:2 * t + 2, :],
                                         h2p[t][:, :, :],
                                         start=(t == 0), stop=(t == DT // 2 - 1),
                                         perf_mode=DRM)
                    with nc.allow_low_precision(reason="y1 fp8 for DR gemm"):
                        nc.scalar.activation(y1p[m // 2][:, m % 2, :], y_ps[:],
                                             AF.Gelu, bias=vecs["b1"][:, m:m + 1])
                _mark(nc, f'B{n}:y2')
                y2p = [by_pool.tile([P, 2, NCH], F8, tag=f"y2p{t}",
                                    name=f"y2p{t}") for t in range(DDT // 2)]
                for m in range(DDT):
                    y_ps = bp.tile([P, NCH], F32, tag="mlp", name="mlp")
                    w3d = w2_t[m][:].rearrange("p (k c) -> p k c", c=P)
                    for t in range(DDT // 2):
                        nc.tensor.matmul(y_ps[:], w3d[:, 2 * t:2 * t + 2, :],
                                         y1p[t][:, :, :],
                                         start=(t == 0), stop=(t == DDT // 2 - 1),
                                         perf_mode=DRM)
                    with nc.allow_low_precision(reason="y2 fp8 for DR gemm"):
                        nc.scalar.activation(y2p[m // 2][:, m % 2, :], y_ps[:],
                                             AF.Gelu, bias=vecs["b2"][:, m:m + 1])
                _mark(nc, f'B{n}:y3')
                for m in range(DT):
                    y_ps = bp.tile([P, NCH], F32, tag="mlp", name="mlp")
                    w3d = w3_t[m][:].rearrange("p (k c) -> p k c", c=P)
                    for t in range(DDT // 2):
                        nc.tensor.matmul(y_ps[:], w3d[:, 2 * t:2 * t + 2, :],
                                         y2p[t][:, :, :],
                                         start=(t == 0), stop=(t == DDT // 2 - 1),
                                         perf_mode=DRM)
                    ot = bw2.tile([P, NCH], F32, tag="ot", name="ot")
                    nc.vector.scalar_tensor_tensor(
                        ot[:], y_ps[:], vecs["b3"][:, m:m + 1], x2_t[m][:],
                        AluOpType.add, AluOpType.add)
                    nc.sync.dma_start(out_d[m * P:(m + 1) * P, cs], ot[:])

            stage_a(0)
            stage_ln(0)
            for n in range(NC):
                if n + 1 < NC:
                    stage_a(n + 1)
                stage_mlp(n)
                if n + 1 < NC:
                    stage_ln(n + 1)
        bhold_cm.__exit__(None, None, None)
        bw_cm.__exit__(None, None, None)


# =========================================================================
# host side
# =========================================================================

def _sinusoidal_pe(seq_len, d_model):
    pos = np.arange(seq_len, dtype=np.float32)[:, None]
    div = np.exp(np.arange(0, d_model, 2, dtype=np.float32)
                 * (-np.log(10000.0) / d_model))
    pe = np.zeros((seq_len, d_model), np.float32)
    pe[:, 0::2] = np.sin(pos * div)
    pe[:, 1::2] = np.cos(pos * div)
    return pe


def _col_block(w):
    """[K, M] -> [M//128 * 128, K] tiles: cb[m*128+p, k*128+c] = w[k*128+p, m*128+c]."""
    K, M = w.shape
    kt, mt = K // P, M // P
    return np.ascontiguousarray(
        w.reshape(kt, P, mt, P).transpose(2, 1, 0, 3).reshape(mt * P, kt * P))


def _vec_tiles(v, ntiles):
    return np.ascontiguousarray(np.asarray(v, np.float32).reshape(ntiles, P).T)


def make_in_maps(inputs, S):
    T = B * S // NCORES
    x = np.asarray(inputs["x"], np.float32)
    # peb = (pe + ln1_b) / ln1_g — the LN1 gain is applied after the pe-add
    # via the ACT copy's scale slot (h = (LN + peb) * g)
    g1 = np.asarray(inputs["ln1_g"], np.float32)
    pe = (_sinusoidal_pe(S, D) + np.asarray(inputs["ln1_b"], np.float32)[None, :]) / g1[None, :]

    indsum = np.zeros((DT * P, 32), np.float32)
    indbc = np.zeros((DT * 32, P), np.float32)
    for t in range(DT):
        for j in range(P):
            h = 2 * t + (1 if j >= 64 else 0)
            indsum[t * P + j, h] = 1.0
            indbc[t * 32 + h, j] = 1.0

    F8NP = ml_dtypes.float8_e4m3
    BFNP = ml_dtypes.bfloat16

    def _pair_pack(w):
        """[K, M] -> [K//2, 2M]: row 128t+p, col j*M+c = w[(2t+j)*128+p, c]."""
        K, M = w.shape
        return np.ascontiguousarray(
            w.reshape(K // 256, 2, P, M).transpose(0, 2, 1, 3).reshape(K // 2, 2 * M))

    wqkv = np.asarray(inputs["w_qkv"], np.float32)
    shared = {
        "wq": _pair_pack(np.ascontiguousarray(wqkv[:, :D])).astype(F8NP),
        "wkv": _pair_pack(np.ascontiguousarray(wqkv[:, D:])).astype(F8NP),
        "wout": _col_block(np.asarray(inputs["w_out"], np.float32)).astype(F8NP),
        "w1": _col_block(np.asarray(inputs["w1"], np.float32)).astype(F8NP),
        "w2": _col_block(np.asarray(inputs["w2"], np.float32)).astype(F8NP),
        "w3": _col_block(np.asarray(inputs["w3"], np.float32)).astype(F8NP),
        "lng1": _vec_tiles(inputs["ln1_g"], DT),
        "lng2": _vec_tiles(inputs["ln2_g"], DT),
        "lnb2": _vec_tiles(inputs["ln2_b"], DT),
        "bout": _vec_tiles(inputs["b_out"], DT),
        "b1": _vec_tiles(inputs["b1"], DDT),
        "b2": _vec_tiles(inputs["b2"], DDT),
        "b3": _vec_tiles(inputs["b3"], DT),
        "ones": np.ones((P, P), np.float32),
        "indsum": indsum.astype(BFNP),
        "indbc": indbc.astype(BFNP),
    }
    in_maps = []
    for c in range(NCORES):
        b, hhalf = divmod(c, NCORES // B)
        s0 = hhalf * T
        m = dict(shared)
        m["xT"] = np.ascontiguousarray(x[b, s0:s0 + T, :].T)
        m["peb"] = np.ascontiguousarray(pe[s0:s0 + T, :].T).astype(BFNP)
        in_maps.append(m)
    return in_maps


def gather(results, S):
    T = B * S // NCORES
    full = np.empty((B, S, D), np.float32)
    for c in range(NCORES):
        b, hhalf = divmod(c, NCORES // B)
        s0 = hhalf * T
        full[b, s0:s0 + T, :] = results[c]["out"].T
    return full


_GRAPH_CACHE = {}


def _get_graph(S):
    T = B * S // NCORES
    if T not in _GRAPH_CACHE:
        _GRAPH_CACHE[T] = build_graph(T)
    return _GRAPH_CACHE[T]


def run(inputs, S, **kw):
    nc = _get_graph(S)
    in_maps = make_in_maps(inputs, S)
    res = run_bass_kernel_spmd(nc, in_maps, core_ids=list(range(NCORES)), **kw)
    return gather(res.results, S), res


def kernel(**inputs):
    out, _ = run(inputs, S_FULL)
    return out


# revision 28
# speedup vs baseline: 1.2244x; 1.2244x over previous
"""Trainium2 Bass kernel for nn_Attn_86784109183632.

Transformer block: LN1 -> +sinusoidal PE -> linear (efficient) attention ->
w_out + residual -> LN2 -> 3-layer gelu MLP + residual.
B=4, S=4096, D=1024, H=16, dh=64.

Sharding: data-parallel over (batch, seq-half) -> 8 cores x 2048 tokens.
The only cross-core term is the k-softmax normalizer and k^T v context
(sums over the sequence axis), reduced with a tiny pairwise AllReduce
([128,1024] fp32) between the two cores holding the same batch, overlapped
with the q projection + q softmax.

Precision plan (tolerance is rel-max 2e-2; measured ~1.3e-2 in numpy sim):
  - x / residuals / LN statistics: fp32(r)
  - attention path (h, w_qkv, exp(k), v, softmaxed q, ctx): bf16
  - MLP + w_out GEMMs: fp8 e4m3 with DoubleRow perf mode (2 k-blocks per
    matmul pass), weights SBUF-resident, activations quantized on the fly
    by the Activation engine into paired [128, 2, 512] tiles.
Softmaxed q stays SBUF-resident between the q phase and phase B (no DRAM
spill). All activations are dim-major [dims, tokens].
"""

import sys

if "/opt/trn_rl_repo" not in sys.path:
    sys.path.insert(0, "/opt/trn_rl_repo")

import ml_dtypes
import numpy as np

import concourse.mybir as mybir
import concourse.tile as tile
from concourse import bacc
from concourse.alu_op_type import AluOpType
from concourse.bass_utils import run_bass_kernel_spmd

P = 128
D = 1024
DD = 2048  # mlp hidden
H = 16
DH = 64
B = 4
S_FULL = 4096
NCORES = 8
EPS = 1e-6

FR = mybir.dt.float32r
F32 = mybir.dt.float32
BF = mybir.dt.bfloat16
F8 = mybir.dt.float8e4
AF = mybir.ActivationFunctionType
DRM = mybir.MatmulPerfMode.DoubleRow

DT = D // P        # 8 d-tiles
DDT = DD // P      # 16 mlp-tiles
NCH = 512          # token chunk (one fp32 psum bank)


def _ctx_col(i):
    """Free-dim offset of head-pair block i inside ctx psum (4 pairs/bank)."""
    return 512 * (i // 4) + 65 * (i % 4)


def build_graph(T):
    """Build the SPMD graph for T tokens per core. T % 512 == 0."""
    assert T % NCH == 0
    TT = T // P           # token tiles
    NC = T // NCH         # token chunks

    nc = bacc.Bacc("TRN2", target_bir_lowering=False, debug=False,
                   num_devices=NCORES)

    tn = {}
    tn["xT"] = nc.dram_tensor("xT", [D, T], FR, kind="ExternalInput")
    tn["peb"] = nc.dram_tensor("peb", [D, T], BF, kind="ExternalInput")
    # q/kv weights pair-packed for DoubleRow: row 128t+p, col j*M+c holds
    # w[(2t+j)*128+p, c]
    tn["wq"] = nc.dram_tensor("wq", [D // 2, 2 * D], F8, kind="ExternalInput")
    tn["wkv"] = nc.dram_tensor("wkv", [D // 2, 4 * D], F8, kind="ExternalInput")
    tn["wout"] = nc.dram_tensor("wout", [D, D], F8, kind="ExternalInput")  # col-block
    tn["w1"] = nc.dram_tensor("w1", [DD, D], F8, kind="ExternalInput")  # col-block
    tn["w2"] = nc.dram_tensor("w2", [DD, DD], F8, kind="ExternalInput")  # col-block
    tn["w3"] = nc.dram_tensor("w3", [D, DD], F8, kind="ExternalInput")  # col-block
    # per-dim vectors laid out [128, n_tiles] (column t = dims 128t..128t+127)
    for name, nt in [("lng1", DT), ("lng2", DT), ("lnb2", DT),
                     ("bout", DT), ("b1", DDT), ("b2", DDT), ("b3", DT)]:
        tn[name] = nc.dram_tensor(name, [P, nt], F32, kind="ExternalInput")
    tn["ones"] = nc.dram_tensor("ones", [P, P], FR, kind="ExternalInput")
    tn["indsum"] = nc.dram_tensor("indsum", [DT * P, 32], BF, kind="ExternalInput")
    tn["indbc"] = nc.dram_tensor("indbc", [DT * 32, P], BF, kind="ExternalInput")
    tn["out"] = nc.dram_tensor("out", [D, T], F32, kind="ExternalOutput")

    PHASE_MARKS.clear()
    with tile.TileContext(nc) as tc:
        _build_body(nc, tc, T, TT, NC, tn)
    import json as _json
    _json.dump(PHASE_MARKS, open("/tmp/phase_marks.json", "w"))
    nc.compile()
    return nc


PHASE_MARKS = []


def _mark(nc, label):
    PHASE_MARKS.append((label, nc.next_id()))


def _build_body(nc, tc, T, TT, NC, tn):
    xT, out_d = tn["xT"], tn["out"]
    RG = [[0, 1], [2, 3], [4, 5], [6, 7]]

    with tc.tile_pool(name="const", bufs=1) as const, \
         tc.tile_pool(name="ctx_hold", bufs=1) as ctx_hold, \
         tc.tile_pool(name="qs_hold", bufs=1) as qs_hold, \
         tc.tile_pool(name="dram", bufs=1, space="DRAM") as dram:

        # ------------- constants -------------
        ones_t = const.tile([P, P], FR)
        nc.sync.dma_start(ones_t[:], tn["ones"][:])
        indsum_t, indbc_t = [], []
        for t in range(DT):
            it = const.tile([P, 32], BF, tag=f"indsum{t}", name=f"indsum{t}")
            nc.sync.dma_start(it[:], tn["indsum"][t * P:(t + 1) * P, :])
            indsum_t.append(it)
            bt = const.tile([32, P], BF, tag=f"indbc{t}", name=f"indbc{t}")
            nc.sync.dma_start(bt[:], tn["indbc"][t * 32:(t + 1) * 32, :])
            indbc_t.append(bt)
        vecs = {}
        for name, nt in [("lng1", DT), ("lng2", DT), ("lnb2", DT),
                         ("bout", DT), ("b1", DDT), ("b2", DDT), ("b3", DT)]:
            v = const.tile([P, nt], F32, tag=name)
            nc.sync.dma_start(v[:], tn[name][:])
            vecs[name] = v

        eps_t = const.tile([P, 1], F32, tag="eps", name="eps")
        nc.vector.memset(eps_t[:], EPS)
        ctxg_sb = ctx_hold.tile([P, 1024], F32)   # ctx after AllReduce

        # softmaxed q, SBUF-resident through phase B
        qs_sb = [qs_hold.tile([P, T], BF, tag=f"qs{m}", name=f"qs{m}")
                 for m in range(DT)]

        ar_in = dram.tile([P, 1024], F32, tag="ar_in", name="ar_in")
        ar_out = dram.tile([P, 1024], F32, tag="ar_out", name="ar_out")

        def ln_stats_to_scales(mu_ps, ms_ps, pool, tagsfx, tmp_pool=None):
            """mu_ps/ms_ps: psum [P, NCH] broadcast sums of x and x^2 over D.
            Returns (rstd_b, murstd_b) SBUF [P, NCH] f32."""
            tpool = tmp_pool if tmp_pool is not None else pool
            mu_n = tpool.tile([P, NCH], F32, tag="t_mun" + tagsfx)
            nc.scalar.mul(mu_n[:], mu_ps[:], 1.0 / D)
            var = tpool.tile([P, NCH], F32, tag="t_var" + tagsfx)
            nc.scalar.activation(var[:], mu_ps[:], AF.Square, scale=1.0 / D)
            ex2 = tpool.tile([P, NCH], F32, tag="t_ex2" + tagsfx)
            nc.scalar.mul(ex2[:], ms_ps[:], 1.0 / D)
            nc.vector.tensor_sub(var[:], ex2[:], var[:])
            sd = tpool.tile([P, NCH], F32, tag="t_sd" + tagsfx)
            nc.scalar.activation(sd[:], var[:], AF.Sqrt, bias=eps_t[:])
            rb = pool.tile([P, NCH], F32, tag="rb" + tagsfx)
            nc.vector.reciprocal(rb[:], sd[:])
            mb = pool.tile([P, NCH], F32, tag="mb" + tagsfx)
            nc.vector.tensor_mul(mb[:], mu_n[:], rb[:])
            return rb, mb

        # =================================================================
        # PHASE A
        # =================================================================
        with tc.tile_pool(name="h_pool", bufs=1) as h_pool, \
             tc.tile_pool(name="expq_pool", bufs=1) as eqp:
            # h as fp8 k-pair tiles for DoubleRow: h_p[t][:, j, s] = h block 2t+j
            h_p = [h_pool.tile([P, 2, T], F8, tag=f"h{t}", name=f"h{t}")
                   for t in range(DT // 2)]
            expq_a = [[None] * DT for _ in range(NC)]

            # ---- fused: LN1 + kv-GEMM + ctx + q-GEMM, chunk-pipelined ----
            with tc.tile_pool(name="wkv_pool", bufs=1) as wkv_pool, \
                 tc.tile_pool(name="ln1_work", bufs=2) as lnw, \
                 tc.tile_pool(name="ln1_tmp", bufs=1) as lntmp, \
                 tc.tile_pool(name="ln1_x", bufs=2) as lnx, \
                 tc.tile_pool(name="ln1_stream", bufs=2) as lns, \
                 tc.tile_pool(name="kv_work", bufs=2) as kvw, \
                 tc.tile_pool(name="kv_ev", bufs=1) as kvev, \
                 tc.tile_pool(name="ln1_psum", bufs=1, space="PSUM") as lnp, \
                 tc.tile_pool(name="mm_psum", bufs=2, space="PSUM") as kvp_pool, \
                 tc.tile_pool(name="q_psum", bufs=2, space="PSUM") as qp_pool, \
                 tc.tile_pool(name="ctx_psum", bufs=1, space="PSUM") as ctxp_pool:
                ctx_ps = ctxp_pool.tile([P, 1024], F32, tag="ctx", name="ctx")
                wkv_t, wq_t = [], []

                def load_w():
                    for t in range(DT // 2):
                        wt = wkv_pool.tile([P, 2, 2 * D], F8, tag=f"wkv{t}",
                                           name=f"wkv{t}")
                        nc.sync.dma_start(
                            wt[:].rearrange("p j c -> p (j c)"),
                            tn["wkv"][t * P:(t + 1) * P, :])
                        wkv_t.append(wt)
                        qt = wkv_pool.tile([P, 2, D], F8, tag=f"wq{t}",
                                           name=f"wq{t}")
                        nc.sync.dma_start(
                            qt[:].rearrange("p j c -> p (j c)"),
                            tn["wq"][t * P:(t + 1) * P, :])
                        wq_t.append(qt)

                stats_ps = {}
                xcur = {}

                def ln1_stats(c):
                    cs = slice(c * NCH, (c + 1) * NCH)
                    mu = lnp.tile([P, NCH], F32, tag="mu", name="mu")
                    ms = lnp.tile([P, NCH], F32, tag="ms", name="ms")
                    xs = []
                    for k in range(DT):
                        xk = lnx.tile([P, NCH], FR, tag=f"xc{k}", name=f"xc{k}")
                        nc.sync.dma_start(xk[:], xT[k * P:(k + 1) * P, cs])
                        sq = lns.tile([P, NCH], FR, tag="sq", name="sq")
                        nc.scalar.activation(sq[:], xk[:], AF.Square)
                        nc.tensor.matmul(mu[:], ones_t[:], xk[:],
                                         start=(k == 0), stop=(k == DT - 1))
                        nc.tensor.matmul(ms[:], ones_t[:], sq[:],
                                         start=(k == 0), stop=(k == DT - 1))
                        xs.append(xk)
                    stats_ps[c] = (mu, ms)
                    xcur[c] = xs

                def ln1_apply(c):
                    cs = slice(c * NCH, (c + 1) * NCH)
                    mu, ms = stats_ps.pop(c)
                    rb, mb = ln_stats_to_scales(mu, ms, lnw, "1", lntmp)
                    xs = xcur.pop(c)
                    for k in range(DT):
                        pk = lns.tile([P, NCH], BF, tag="pe", name="pe")
                        nc.sync.dma_start(pk[:], tn["peb"][k * P:(k + 1) * P, cs])
                        tmp = lns.tile([P, NCH], F32, tag="lnt", name="lnt")
                        nc.vector.tensor_mul(tmp[:], xs[k][:], rb[:])
                        nc.vector.tensor_sub(tmp[:], tmp[:], mb[:])
                        # h = (LN1(x) + (pe + b)/g) * g — gain applied via the
                        # ACT copy's scale slot; host sends peb = (pe + b)/g
                        nc.vector.tensor_add(tmp[:], tmp[:], pk[:])
                        with nc.allow_low_precision(reason="h fp8 for DR gemms"):
                            nc.scalar.activation(h_p[k // 2][:, k % 2, cs],
                                                 tmp[:], AF.Identity,
                                                 scale=vecs["lng1"][:, k:k + 1])

                pending = []  # (ek, vv, global_tt) awaiting ctx matmuls

                def flush_ctx(last=False):
                    while pending:
                        ek, vv, pt = pending.pop(0)
                        for h16 in range(H):
                            i, j = h16 // 2, h16 % 2
                            c0 = _ctx_col(i)
                            nc.tensor.matmul(
                                ctx_ps[64 * j:64 * j + 64, c0:c0 + 65],
                                ek[:, 64 * h16:64 * h16 + 64],
                                vv[:, h16 * 65:(h16 + 1) * 65],
                                start=(pt == 0 and h16 in (0, 1, 8, 9)),
                                stop=(pt == TT - 1 and h16 in (6, 7, 14, 15)))

                def kv_ctx(c):
                    for lt in range(NCH // P):
                        tt = c * (NCH // P) + lt
                        ts_ = slice(tt * P, (tt + 1) * P)
                        ek = kvw.tile([P, D], BF, tag="ek", name="ek")
                        vv = kvw.tile([P, H * 65], BF, tag="vv", name="vv")
                        vv3 = vv[:].rearrange("p (h e) -> p h e", e=65)
                        for n in range(4):
                            pn = kvp_pool.tile([P, 512], F32, tag="kv", name="kv")
                            for t in range(DT // 2):
                                nc.tensor.matmul(
                                    pn[:], h_p[t][:, :, ts_],
                                    wkv_t[t][:, :, n * 512:(n + 1) * 512],
                                    start=(t == 0), stop=(t == DT // 2 - 1),
                                    perf_mode=DRM)
                            if n < 2:
                                nc.scalar.activation(ek[:, n * 512:(n + 1) * 512],
                                                     pn[:], AF.Exp)
                            else:
                                nc.vector.tensor_copy(
                                    vv3[:, (n - 2) * 8:(n - 1) * 8, 0:64],
                                    pn[:].rearrange("p (h e) -> p h e", e=64))
                        nc.vector.memset(vv3[:, :, 64:65], 1.0)
                        flush_ctx()
                        pending.append((ek, vv, tt))

                def qexp(c):
                    cs = slice(c * NCH, (c + 1) * NCH)
                    for m in range(DT):
                        qp = qp_pool.tile([P, NCH], F32, tag="q", name="q")
                        for t in range(DT // 2):
                            nc.tensor.matmul(
                                qp[:], wq_t[t][:, :, m * P:(m + 1) * P],
                                h_p[t][:, :, cs],
                                start=(t == 0), stop=(t == DT // 2 - 1),
                                perf_mode=DRM)
                        eq = eqp.tile([P, NCH], BF, tag=f"expq{c}_{m}",
                                      name=f"expq{c}_{m}")
                        nc.scalar.activation(eq[:], qp[:], AF.Exp)
                        expq_a[c][m] = eq

                _mark(nc, 'A:start')
                load_w()
                ln1_stats(0)
                for c in range(NC):
                    # apply(c) first: it drains the single-buffered stats psum
                    # before stats(c+1) reuses it
                    ln1_apply(c)
                    if c + 1 < NC:
                        ln1_stats(c + 1)
                    kv_ctx(c)
                flush_ctx(last=True)

                _mark(nc, 'A:ctx_evict')
                ctx_sb = kvev.tile([P, 1024], F32, tag="ctxev", name="ctxev")
                nc.vector.tensor_copy(ctx_sb[:], ctx_ps[:])
                nc.sync.dma_start(ar_in[:], ctx_sb[:])

                # q-GEMM after the evict in program order: the scheduler
                # pulls it forward into main-loop bubbles, and whatever is
                # left covers the AllReduce latency window.
                for c in range(NC):
                    qexp(c)

            nc.gpsimd.collective_compute(
                "AllReduce", AluOpType.add, replica_groups=RG,
                ins=[ar_in[:].opt()], outs=[ar_out[:].opt()])
            nc.sync.dma_start(ctxg_sb[:], ar_out[:])

            _mark(nc, 'A:q')
            # ---------- softmax tail: 1/sum + broadcast, overlaps AllReduce
            with tc.tile_pool(name="q_small", bufs=3) as qsm, \
                 tc.tile_pool(name="bc_psum", bufs=2, space="PSUM") as bc_pool, \
                 tc.tile_pool(name="ssum_psum", bufs=2, space="PSUM") as sp_pool:
                for c in range(NC):
                    cs = slice(c * NCH, (c + 1) * NCH)
                    expq = expq_a[c]
                    s_ps = sp_pool.tile([32, NCH], F32, tag="ssum", name="ssum")
                    for m in range(DT):
                        nc.tensor.matmul(s_ps[:], indsum_t[m][:], expq[m][:],
                                         start=(m == 0), stop=(m == DT - 1))
                    rs = qsm.tile([32, NCH], BF, tag="recS", name="recS")
                    nc.vector.tensor_copy(rs[:], s_ps[:])
                    with nc.allow_low_precision(reason="softmax scale in bf16"):
                        nc.vector.reciprocal(rs[0:H, :], s_ps[0:H, :])
                    for m in range(DT):
                        bc = bc_pool.tile([P, NCH], F32, tag="bc", name="bc")
                        nc.tensor.matmul(bc[:], indbc_t[m][:], rs[:],
                                         start=True, stop=True)
                        with nc.allow_low_precision(reason="softmaxed q in bf16"):
                            nc.vector.tensor_mul(qs_sb[m][:, cs], expq[m][:], bc[:])

        # phase-B weights, fp8, SBUF-resident (loaded once at B start; the
        # first consumers — wout of chunk 0 — need only the first 1 MB).
        bw_cm = tc.tile_pool(name="bw_hold", bufs=1)
        bw = bw_cm.__enter__()
        wout_t = [bw.tile([P, D], F8, tag=f"wo{m}", name=f"wo{m}")
                  for m in range(DT)]
        w1_t = [bw.tile([P, D], F8, tag=f"w1_{m}", name=f"w1_{m}")
                for m in range(DDT)]
        w2_t = [bw.tile([P, DD], F8, tag=f"w2_{m}", name=f"w2_{m}")
                for m in range(DDT)]
        w3_t = [bw.tile([P, DD], F8, tag=f"w3_{m}", name=f"w3_{m}")
                for m in range(DT)]
        for m in range(DT):
            nc.sync.dma_start(wout_t[m][:], tn["wout"][m * P:(m + 1) * P, :])
        for m in range(DDT):
            nc.sync.dma_start(w1_t[m][:], tn["w1"][m * P:(m + 1) * P, :])
        for m in range(DDT):
            nc.sync.dma_start(w2_t[m][:], tn["w2"][m * P:(m + 1) * P, :])
        for m in range(DT):
            nc.sync.dma_start(w3_t[m][:], tn["w3"][m * P:(m + 1) * P, :])

        _mark(nc, 'ctxnorm')
        # normalize ctx into block-diagonal head-pair lhsT tiles (bf16):
        # ctxd[:, 128i:128(i+1)] = [[ctx_{2i}*zr, 0], [0, ctx_{2i+1}*zr]]
        bhold_cm = tc.tile_pool(name="b_hold", bufs=1)
        bhold = bhold_cm.__enter__()
        ctxd_sb = bhold.tile([P, 1024], BF, tag="ctxd", name="ctxd")
        zr_sb = bhold.tile([P, 8], F32, tag="zr", name="zr")
        for i in range(8):
            c0 = _ctx_col(i)
            nc.vector.reciprocal(zr_sb[:, i:i + 1], ctxg_sb[:, c0 + 64:c0 + 65])
        nc.scalar.mul(zr_sb[:], zr_sb[:], DH ** -0.5)
        nc.vector.memset(ctxd_sb[:], 0.0)
        for h16 in range(H):
            i, j = h16 // 2, h16 % 2
            c0 = _ctx_col(i)
            nc.vector.tensor_scalar(
                ctxd_sb[64 * j:64 * j + 64, 128 * i + 64 * j:128 * i + 64 * j + 64],
                ctxg_sb[64 * j:64 * j + 64, c0:c0 + 64],
                zr_sb[64 * j:64 * j + 64, i:i + 1], None, AluOpType.mult)

        # =================================================================
        # PHASE B: per token chunk attn -> w_out+res -> LN2 -> MLP+res
        # (cross-chunk pipelined; MLP + w_out GEMMs in fp8 DoubleRow)
        # =================================================================
        with tc.tile_pool(name="b_attp", bufs=1) as bap, \
             tc.tile_pool(name="b_act2", bufs=2) as bact2, \
             tc.tile_pool(name="b_stream", bufs=4) as bstr, \
             tc.tile_pool(name="b_y", bufs=1) as by_pool, \
             tc.tile_pool(name="b_work", bufs=2) as bw2, \
             tc.tile_pool(name="b_lnw", bufs=1) as blnw, \
             tc.tile_pool(name="b_psum", bufs=2, space="PSUM") as bp, \
             tc.tile_pool(name="b_stat_psum", bufs=1, space="PSUM") as bsp:
            x2_c = {}
            h2_c = {}
            stats_c = {}

            def stage_a(n):
                cs = slice(n * NCH, (n + 1) * NCH)
                _mark(nc, f'B{n}:attn')
                att_p = [bap.tile([P, 2, NCH], F8, tag=f"attp{t}",
                                  name=f"attp{t}") for t in range(DT // 2)]
                for i in range(DT):
                    ap_ps = bp.tile([P, NCH], F32, tag="attn", name="attn")
                    nc.tensor.matmul(ap_ps[:], ctxd_sb[:, P * i:P * (i + 1)],
                                     qs_sb[i][:, cs], start=True, stop=True)
                    with nc.allow_low_precision(reason="attn out fp8 for DR gemm"):
                        nc.scalar.mul(att_p[i // 2][:, i % 2, :], ap_ps[:], 1.0)
                _mark(nc, f'B{n}:wout')
                x2_t = []
                mu2 = bsp.tile([P, NCH], F32, tag="mu2", name="mu2")
                ms2 = bsp.tile([P, NCH], F32, tag="ms2", name="ms2")
                for m in range(DT):
                    wo_ps = bp.tile([P, NCH], F32, tag="wout", name="wout")
                    w3d = wout_t[m][:].rearrange("p (k c) -> p k c", c=P)
                    for t in range(DT // 2):
                        nc.tensor.matmul(wo_ps[:], w3d[:, 2 * t:2 * t + 2, :],
                                         att_p[t][:, :, :],
                                         start=(t == 0), stop=(t == DT // 2 - 1),
                                         perf_mode=DRM)
                    xc = bw2.tile([P, NCH], FR, tag="xc", name="xc")
                    nc.sync.dma_start(xc[:], xT[m * P:(m + 1) * P, cs])
                    x2 = bact2.tile([P, NCH], FR, tag=f"x2_{m}", name=f"x2_{m}")
                    nc.vector.scalar_tensor_tensor(
                        x2[:], wo_ps[:], vecs["bout"][:, m:m + 1], xc[:],
                        AluOpType.add, AluOpType.add)
                    x2_t.append(x2)
                    sq = bw2.tile([P, NCH], FR, tag="sq2", name="sq2")
                    nc.scalar.activation(sq[:], x2[:], AF.Square)
                    nc.tensor.matmul(mu2[:], ones_t[:], x2[:],
                                     start=(m == 0), stop=(m == DT - 1))
                    nc.tensor.matmul(ms2[:], ones_t[:], sq[:],
                                     start=(m == 0), stop=(m == DT - 1))
                x2_c[n] = x2_t
                stats_c[n] = (mu2, ms2)

            def stage_ln(n):
                _mark(nc, f'B{n}:ln2')
                mu2, ms2 = stats_c.pop(n)
                rstd, murstd = ln_stats_to_scales(mu2, ms2, blnw, "2")
                h2p = [bact2.tile([P, 2, NCH], F8, tag=f"h2p{t}",
                                  name=f"h2p{t}") for t in range(DT // 2)]
                for m in range(DT):
                    tmp = bw2.tile([P, NCH], F32, tag="h2t", name="h2t")
                    nc.vector.tensor_mul(tmp[:], x2_c[n][m][:], rstd[:])
                    nc.vector.tensor_sub(tmp[:], tmp[:], murstd[:])
                    # h2 = tmp*g2 + b2ln, quantized to fp8 on the ACT engine
                    with nc.allow_low_precision(reason="h2 fp8 for DR gemm"):
                        nc.scalar.activation(
                            h2p[m // 2][:, m % 2, :], tmp[:], AF.Identity,
                            scale=vecs["lng2"][:, m:m + 1],
                            bias=vecs["lnb2"][:, m:m + 1])
                h2_c[n] = h2p

            def stage_mlp(n):
                cs = slice(n * NCH, (n + 1) * NCH)
                h2p = h2_c.pop(n)
                x2_t = x2_c.pop(n)
                _mark(nc, f'B{n}:y1')
                y1p = [by_pool.tile([P, 2, NCH], F8, tag=f"y1p{t}",
                                    name=f"y1p{t}") for t in range(DDT // 2)]
                for m in range(DDT):
                    y_ps = bp.tile([P, NCH], F32, tag="mlp", name="mlp")
                    w3d = w1_t[m][:].rearrange("p (k c) -> p k c", c=P)
                    for t in range(DT // 2):
                        nc.tensor.matmul(y_ps[:], w3d[:, 2 * t:2 * t + 2, :],
                                         h2p[t][:, :, :],
                                         start=(t == 0), stop=(t == DT // 2 - 1),
                                         perf_mode=DRM)
                    with nc.allow_low_precision(reason="y1 fp8 for DR gemm"):
                        nc.scalar.activation(y1p[m // 2][:, m % 2, :], y_ps[:],
                                             AF.Gelu, bias=vecs["b1"][:, m:m + 1])
                _mark(nc, f'B{n}:y2')
                y2p = [by_pool.tile([P, 2, NCH], F8, tag=f"y2p{t}",
                                    name=f"y2p{t}") for t in range(DDT // 2)]
                for m in range(DDT):
                    y_ps = bp.tile([P, NCH], F32, tag="mlp", name="mlp")
                    w3d = w2_t[m][:].rearrange("p (k c) -> p k c", c=P)
                    for t in range(DDT // 2):
                        nc.tensor.matmul(y_ps[:], w3d[:, 2 * t:2 * t + 2, :],
                                         y1p[t][:, :, :],
                                         start=(t == 0), stop=(t == DDT // 2 - 1),
                                         perf_mode=DRM)
                    with nc.allow_low_precision(reason="y2 fp8 for DR gemm"):
                        nc.scalar.activation(y2p[m // 2][:, m % 2, :], y_ps[:],
                                             AF.Gelu, bias=vecs["b2"][:, m:m + 1])
                _mark(nc, f'B{n}:y3')
                for m in range(DT):
                    y_ps = bp.tile([P, NCH], F32, tag="mlp", name="mlp")
                    w3d = w3_t[m][:].rearrange("p (k c) -> p k c", c=P)
                    for t in range(DDT // 2):
                        nc.tensor.matmul(y_ps[:], w3d[:, 2 * t:2 * t + 2, :],
                                         y2p[t][:, :, :],
                                         start=(t == 0), stop=(t == DDT // 2 - 1),
                                         perf_mode=DRM)
                    ot = bw2.tile([P, NCH], F32, tag="ot", name="ot")
                    nc.vector.scalar_tensor_tensor(
                        ot[:], y_ps[:], vecs["b3"][:, m:m + 1], x2_t[m][:],
                        AluOpType.add, AluOpType.add)
                    nc.sync.dma_start(out_d[m * P:(m + 1) * P, cs], ot[:])

            stage_a(0)
            stage_ln(0)
            for n in range(NC):
                if n + 1 < NC:
                    stage_a(n + 1)
                stage_mlp(n)
                if n + 1 < NC:
                    stage_ln(n + 1)
        bhold_cm.__exit__(None, None, None)
        bw_cm.__exit__(None, None, None)


# =========================================================================
# host side
# =========================================================================

def _sinusoidal_pe(seq_len, d_model):
    pos = np.arange(seq_len, dtype=np.float32)[:, None]
    div = np.exp(np.arange(0, d_model, 2, dtype=np.float32)
                 * (-np.log(10000.0) / d_model))
    pe = np.zeros((seq_len, d_model), np.float32)
    pe[:, 0::2] = np.sin(pos * div)
    pe[:, 1::2] = np.cos(pos * div)
    return pe


def _col_block(w):
    """[K, M] -> [M//128 * 128, K] tiles: cb[m*128+p, k*128+c] = w[k*128+p, m*128+c]."""
    K, M = w.shape
    kt, mt = K // P, M // P
    return np.ascontiguousarray(
        w.reshape(kt, P, mt, P).transpose(2, 1, 0, 3).reshape(mt * P, kt * P))


def _vec_tiles(v, ntiles):
    return np.ascontiguousarray(np.asarray(v, np.float32).reshape(ntiles, P).T)


def make_in_maps(inputs, S):
    T = B * S // NCORES
    x = np.asarray(inputs["x"], np.float32)
    # peb = (pe + ln1_b) / ln1_g — the LN1 gain is applied after the pe-add
    # via the ACT copy's scale slot (h = (LN + peb) * g)
    g1 = np.asarray(inputs["ln1_g"], np.float32)
    pe = (_sinusoidal_pe(S, D) + np.asarray(inputs["ln1_b"], np.float32)[None, :]) / g1[None, :]

    indsum = np.zeros((DT * P, 32), np.float32)
    indbc = np.zeros((DT * 32, P), np.float32)
    for t in range(DT):
        for j in range(P):
            h = 2 * t + (1 if j >= 64 else 0)
            indsum[t * P + j, h] = 1.0
            indbc[t * 32 + h, j] = 1.0

    F8NP = ml_dtypes.float8_e4m3
    BFNP = ml_dtypes.bfloat16

    def _pair_pack(w):
        """[K, M] -> [K//2, 2M]: row 128t+p, col j*M+c = w[(2t+j)*128+p, c]."""
        K, M = w.shape
        return np.ascontiguousarray(
            w.reshape(K // 256, 2, P, M).transpose(0, 2, 1, 3).reshape(K // 2, 2 * M))

    wqkv = np.asarray(inputs["w_qkv"], np.float32)
    shared = {
        "wq": _pair_pack(np.ascontiguousarray(wqkv[:, :D])).astype(F8NP),
        "wkv": _pair_pack(np.ascontiguousarray(wqkv[:, D:])).astype(F8NP),
        "wout": _col_block(np.asarray(inputs["w_out"], np.float32)).astype(F8NP),
        "w1": _col_block(np.asarray(inputs["w1"], np.float32)).astype(F8NP),
        "w2": _col_block(np.asarray(inputs["w2"], np.float32)).astype(F8NP),
        "w3": _col_block(np.asarray(inputs["w3"], np.float32)).astype(F8NP),
        "lng1": _vec_tiles(inputs["ln1_g"], DT),
        "lng2": _vec_tiles(inputs["ln2_g"], DT),
        "lnb2": _vec_tiles(inputs["ln2_b"], DT),
        "bout": _vec_tiles(inputs["b_out"], DT),
        "b1": _vec_tiles(inputs["b1"], DDT),
        "b2": _vec_tiles(inputs["b2"], DDT),
        "b3": _vec_tiles(inputs["b3"], DT),
        "ones": np.ones((P, P), np.float32),
        "indsum": indsum.astype(BFNP),
        "indbc": indbc.astype(BFNP),
    }
    in_maps = []
    for c in range(NCORES):
        b, hhalf = divmod(c, NCORES // B)
        s0 = hhalf * T
        m = dict(shared)
        m["xT"] = np.ascontiguousarray(x[b, s0:s0 + T, :].T)
        m["peb"] = np.ascontiguousarray(pe[s0:s0 + T, :].T).astype(BFNP)
        in_maps.append(m)
    return in_maps


def gather(results, S):
    T = B * S // NCORES
    full = np.empty((B, S, D), np.float32)
    for c in range(NCORES):
        b, hhalf = divmod(c, NCORES // B)
        s0 = hhalf * T
        full[b, s0:s0 + T, :] = results[c]["out"].T
    return full


_GRAPH_CACHE = {}


def _get_graph(S):
    T = B * S // NCORES
    if T not in _GRAPH_CACHE:
        _GRAPH_CACHE[T] = build_graph(T)
    return _GRAPH_CACHE[T]


def run(inputs, S, **kw):
    nc = _get_graph(S)
    in_maps = make_in_maps(inputs, S)
    res = run_bass_kernel_spmd(nc, in_maps, core_ids=list(range(NCORES)), **kw)
    return gather(res.results, S), res


def kernel(**inputs):
    out, _ = run(inputs, S_FULL)
    return out


# revision 35
# speedup vs baseline: 1.2455x; 1.0172x over previous
"""Trainium2 Bass kernel for nn_Attn_86784109183632.

Transformer block: LN1 -> +sinusoidal PE -> linear (efficient) attention ->
w_out + residual -> LN2 -> 3-layer gelu MLP + residual.
B=4, S=4096, D=1024, H=16, dh=64.

Sharding: data-parallel over (batch, seq-half) -> 8 cores x 2048 tokens.
The only cross-core term is the k-softmax normalizer and k^T v context
(sums over the sequence axis), reduced with a tiny pairwise AllReduce
([128,1024] fp32) between the two cores holding the same batch, overlapped
with the q projection + q softmax.

Precision plan (tolerance is rel-max 2e-2; measured ~1.3e-2 in numpy sim):
  - x / residuals / LN statistics: fp32(r)
  - attention path (h, w_qkv, exp(k), v, softmaxed q, ctx): bf16
  - MLP + w_out GEMMs: fp8 e4m3 with DoubleRow perf mode (2 k-blocks per
    matmul pass), weights SBUF-resident, activations quantized on the fly
    by the Activation engine into paired [128, 2, 512] tiles.
Softmaxed q stays SBUF-resident between the q phase and phase B (no DRAM
spill). All activations are dim-major [dims, tokens].
"""

import sys

if "/opt/trn_rl_repo" not in sys.path:
    sys.path.insert(0, "/opt/trn_rl_repo")

import ml_dtypes
import numpy as np

import concourse.mybir as mybir
import concourse.tile as tile
from concourse import bacc
from concourse.alu_op_type import AluOpType
from concourse.bass_utils import run_bass_kernel_spmd

P = 128
D = 1024
DD = 2048  # mlp hidden
H = 16
DH = 64
B = 4
S_FULL = 4096
NCORES = 8
EPS = 1e-6

FR = mybir.dt.float32r
F32 = mybir.dt.float32
BF = mybir.dt.bfloat16
F8 = mybir.dt.float8e4
AF = mybir.ActivationFunctionType
DRM = mybir.MatmulPerfMode.DoubleRow

DT = D // P        # 8 d-tiles
DDT = DD // P      # 16 mlp-tiles
NCH = 512          # token chunk (one fp32 psum bank)


def _ctx_col(i):
    """Free-dim offset of head-pair block i inside ctx psum (4 pairs/bank)."""
    return 512 * (i // 4) + 65 * (i % 4)


def build_graph(T):
    """Build the SPMD graph for T tokens per core. T % 512 == 0."""
    assert T % NCH == 0
    TT = T // P           # token tiles
    NC = T // NCH         # token chunks

    nc = bacc.Bacc("TRN2", target_bir_lowering=False, debug=False,
                   num_devices=NCORES)

    tn = {}
    tn["xT"] = nc.dram_tensor("xT", [D, T], FR, kind="ExternalInput")
    tn["peb"] = nc.dram_tensor("peb", [D, T], BF, kind="ExternalInput")
    # q/kv weights pair-packed for DoubleRow: row 128t+p, col j*M+c holds
    # w[(2t+j)*128+p, c]
    tn["wq"] = nc.dram_tensor("wq", [D // 2, 2 * D], F8, kind="ExternalInput")
    tn["wkv"] = nc.dram_tensor("wkv", [D // 2, 4 * D], F8, kind="ExternalInput")
    tn["wout"] = nc.dram_tensor("wout", [D, D], F8, kind="ExternalInput")  # col-block
    tn["w1"] = nc.dram_tensor("w1", [DD, D], F8, kind="ExternalInput")  # col-block
    tn["w2"] = nc.dram_tensor("w2", [DD, DD], F8, kind="ExternalInput")  # col-block
    tn["w3"] = nc.dram_tensor("w3", [D, DD], F8, kind="ExternalInput")  # col-block
    # per-dim vectors laid out [128, n_tiles] (column t = dims 128t..128t+127)
    for name, nt in [("lng1", DT), ("lng2", DT), ("lnb2", DT),
                     ("bout", DT), ("b1", DDT), ("b2", DDT), ("b3", DT)]:
        tn[name] = nc.dram_tensor(name, [P, nt], F32, kind="ExternalInput")
    tn["ones"] = nc.dram_tensor("ones", [P, P], FR, kind="ExternalInput")
    tn["indsum"] = nc.dram_tensor("indsum", [DT * P, 32], BF, kind="ExternalInput")
    tn["indbc"] = nc.dram_tensor("indbc", [DT * 32, P], BF, kind="ExternalInput")
    tn["out"] = nc.dram_tensor("out", [D, T], F32, kind="ExternalOutput")

    PHASE_MARKS.clear()
    with tile.TileContext(nc) as tc:
        _build_body(nc, tc, T, TT, NC, tn)
    import json as _json
    _json.dump(PHASE_MARKS, open("/tmp/phase_marks.json", "w"))
    nc.compile()
    return nc


PHASE_MARKS = []


def _mark(nc, label):
    PHASE_MARKS.append((label, nc.next_id()))


def _build_body(nc, tc, T, TT, NC, tn):
    xT, out_d = tn["xT"], tn["out"]
    RG = [[0, 1], [2, 3], [4, 5], [6, 7]]

    with tc.tile_pool(name="const", bufs=1) as const, \
         tc.tile_pool(name="ctx_hold", bufs=1) as ctx_hold, \
         tc.tile_pool(name="qs_hold", bufs=1) as qs_hold, \
         tc.tile_pool(name="dram", bufs=1, space="DRAM") as dram:

        # ------------- constants (tiles now, DMAs deferred so the first
        # x chunk wins the DMA queue) -------------
        ones_t = const.tile([P, P], FR)
        indsum_t, indbc_t = [], []
        for t in range(DT):
            it = const.tile([P, 32], BF, tag=f"indsum{t}", name=f"indsum{t}")
            indsum_t.append(it)
            bt = const.tile([32, P], BF, tag=f"indbc{t}", name=f"indbc{t}")
            indbc_t.append(bt)
        vecs = {}
        for name, nt in [("lng1", DT), ("lng2", DT), ("lnb2", DT),
                         ("bout", DT), ("b1", DDT), ("b2", DDT), ("b3", DT)]:
            vecs[name] = const.tile([P, nt], F32, tag=name, name=name)

        def load_consts():
            nc.sync.dma_start(ones_t[:], tn["ones"][:])
            for t in range(DT):
                nc.sync.dma_start(indsum_t[t][:], tn["indsum"][t * P:(t + 1) * P, :])
                nc.sync.dma_start(indbc_t[t][:], tn["indbc"][t * 32:(t + 1) * 32, :])
            for name in vecs:
                nc.sync.dma_start(vecs[name][:], tn[name][:])

        eps_t = const.tile([P, 1], F32, tag="eps", name="eps")
        nc.vector.memset(eps_t[:], EPS)
        ctxg_sb = ctx_hold.tile([P, 1024], F32)   # ctx after AllReduce

        # softmaxed q, SBUF-resident through phase B
        qs_sb = [qs_hold.tile([P, T], BF, tag=f"qs{m}", name=f"qs{m}")
                 for m in range(DT)]

        ar_in = dram.tile([P, 1024], F32, tag="ar_in", name="ar_in")
        ar_out = dram.tile([2 * P, 1024], F32, tag="ar_out", name="ar_out")

        def ln_stats_to_scales(mu_ps, ms_ps, pool, tagsfx, tmp_pool=None):
            """mu_ps/ms_ps: psum [P, NCH] broadcast sums of x and x^2 over D.
            Returns (rstd_b, murstd_b) SBUF [P, NCH] f32."""
            tpool = tmp_pool if tmp_pool is not None else pool
            mu_n = tpool.tile([P, NCH], F32, tag="t_mun" + tagsfx)
            nc.scalar.mul(mu_n[:], mu_ps[:], 1.0 / D)
            var = tpool.tile([P, NCH], F32, tag="t_var" + tagsfx)
            nc.scalar.activation(var[:], mu_ps[:], AF.Square, scale=1.0 / D)
            ex2 = tpool.tile([P, NCH], F32, tag="t_ex2" + tagsfx)
            nc.scalar.mul(ex2[:], ms_ps[:], 1.0 / D)
            nc.vector.tensor_sub(var[:], ex2[:], var[:])
            sd = tpool.tile([P, NCH], F32, tag="t_sd" + tagsfx)
            nc.scalar.activation(sd[:], var[:], AF.Sqrt, bias=eps_t[:])
            rb = pool.tile([P, NCH], F32, tag="rb" + tagsfx)
            nc.vector.reciprocal(rb[:], sd[:])
            mb = pool.tile([P, NCH], F32, tag="mb" + tagsfx)
            nc.vector.tensor_mul(mb[:], mu_n[:], rb[:])
            return rb, mb

        # =================================================================
        # PHASE A
        # =================================================================
        with tc.tile_pool(name="h_pool", bufs=1) as h_pool, \
             tc.tile_pool(name="expq_pool", bufs=1) as eqp:
            # h as fp8 k-pair tiles for DoubleRow: h_p[t][:, j, s] = h block 2t+j
            h_p = [h_pool.tile([P, 2, T], F8, tag=f"h{t}", name=f"h{t}")
                   for t in range(DT // 2)]
            expq_a = [[None] * DT for _ in range(NC)]

            # ---- fused: LN1 + kv-GEMM + ctx + q-GEMM, chunk-pipelined ----
            with tc.tile_pool(name="wkv_pool", bufs=1) as wkv_pool, \
                 tc.tile_pool(name="ln1_work", bufs=2) as lnw, \
                 tc.tile_pool(name="ln1_tmp", bufs=1) as lntmp, \
                 tc.tile_pool(name="ln1_x", bufs=2) as lnx, \
                 tc.tile_pool(name="ln1_stream", bufs=2) as lns, \
                 tc.tile_pool(name="kv_work", bufs=2) as kvw, \
                 tc.tile_pool(name="kv_ev", bufs=1) as kvev, \
                 tc.tile_pool(name="ln1_psum", bufs=1, space="PSUM") as lnp, \
                 tc.tile_pool(name="mm_psum", bufs=2, space="PSUM") as kvp_pool, \
                 tc.tile_pool(name="q_psum", bufs=2, space="PSUM") as qp_pool, \
                 tc.tile_pool(name="ctx_psum", bufs=1, space="PSUM") as ctxp_pool:
                ctx_ps = ctxp_pool.tile([P, 1024], F32, tag="ctx", name="ctx")
                wkv_t, wq_t = [], []

                def load_w():
                    for t in range(DT // 2):
                        wt = wkv_pool.tile([P, 2, 2 * D], F8, tag=f"wkv{t}",
                                           name=f"wkv{t}")
                        nc.sync.dma_start(
                            wt[:].rearrange("p j c -> p (j c)"),
                            tn["wkv"][t * P:(t + 1) * P, :])
                        wkv_t.append(wt)
                        qt = wkv_pool.tile([P, 2, D], F8, tag=f"wq{t}",
                                           name=f"wq{t}")
                        nc.sync.dma_start(
                            qt[:].rearrange("p j c -> p (j c)"),
                            tn["wq"][t * P:(t + 1) * P, :])
                        wq_t.append(qt)

                stats_ps = {}
                xcur = {}

                def ln1_x_load(c):
                    cs = slice(c * NCH, (c + 1) * NCH)
                    xs = []
                    for k in range(DT):
                        xk = lnx.tile([P, NCH], FR, tag=f"xc{k}", name=f"xc{k}")
                        nc.sync.dma_start(xk[:], xT[k * P:(k + 1) * P, cs])
                        xs.append(xk)
                    xcur[c] = xs

                def ln1_stats(c):
                    mu = lnp.tile([P, NCH], F32, tag="mu", name="mu")
                    ms = lnp.tile([P, NCH], F32, tag="ms", name="ms")
                    for k in range(DT):
                        xk = xcur[c][k]
                        sq = lns.tile([P, NCH], FR, tag="sq", name="sq")
                        nc.scalar.activation(sq[:], xk[:], AF.Square)
                        nc.tensor.matmul(mu[:], ones_t[:], xk[:],
                                         start=(k == 0), stop=(k == DT - 1))
                        nc.tensor.matmul(ms[:], ones_t[:], sq[:],
                                         start=(k == 0), stop=(k == DT - 1))
                    stats_ps[c] = (mu, ms)

                def ln1_apply(c):
                    cs = slice(c * NCH, (c + 1) * NCH)
                    mu, ms = stats_ps.pop(c)
                    rb, mb = ln_stats_to_scales(mu, ms, lnw, "1", lntmp)
                    xs = xcur.pop(c)
                    for k in range(DT):
                        pk = lns.tile([P, NCH], BF, tag="pe", name="pe")
                        nc.sync.dma_start(pk[:], tn["peb"][k * P:(k + 1) * P, cs])
                        tmp = lns.tile([P, NCH], F32, tag="lnt", name="lnt")
                        nc.vector.tensor_mul(tmp[:], xs[k][:], rb[:])
                        nc.vector.tensor_sub(tmp[:], tmp[:], mb[:])
                        # h = (LN1(x) + (pe + b)/g) * g — gain applied via the
                        # ACT copy's scale slot; host sends peb = (pe + b)/g
                        nc.vector.tensor_add(tmp[:], tmp[:], pk[:])
                        with nc.allow_low_precision(reason="h fp8 for DR gemms"):
                            nc.scalar.activation(h_p[k // 2][:, k % 2, cs],
                                                 tmp[:], AF.Identity,
                                                 scale=vecs["lng1"][:, k:k + 1])

                pending = []  # (ek, vv, global_tt) awaiting ctx matmuls

                def flush_ctx(last=False):
                    while pending:
                        ek, vv, pt = pending.pop(0)
                        for h16 in range(H):
                            i, j = h16 // 2, h16 % 2
                            c0 = _ctx_col(i)
                            nc.tensor.matmul(
                                ctx_ps[64 * j:64 * j + 64, c0:c0 + 65],
                                ek[:, 64 * h16:64 * h16 + 64],
                                vv[:, h16 * 65:(h16 + 1) * 65],
                                start=(pt == 0 and h16 in (0, 1, 8, 9)),
                                stop=(pt == TT - 1 and h16 in (6, 7, 14, 15)))

                def kv_ctx(c):
                    for lt in range(NCH // P):
                        tt = c * (NCH // P) + lt
                        ts_ = slice(tt * P, (tt + 1) * P)
                        ek = kvw.tile([P, D], BF, tag="ek", name="ek")
                        vv = kvw.tile([P, H * 65], BF, tag="vv", name="vv")
                        vv3 = vv[:].rearrange("p (h e) -> p h e", e=65)
                        for n in range(4):
                            pn = kvp_pool.tile([P, 512], F32, tag="kv", name="kv")
                            for t in range(DT // 2):
                                nc.tensor.matmul(
                                    pn[:], h_p[t][:, :, ts_],
                                    wkv_t[t][:, :, n * 512:(n + 1) * 512],
                                    start=(t == 0), stop=(t == DT // 2 - 1),
                                    perf_mode=DRM)
                            if n < 2:
                                nc.scalar.activation(ek[:, n * 512:(n + 1) * 512],
                                                     pn[:], AF.Exp)
                            else:
                                nc.vector.tensor_copy(
                                    vv3[:, (n - 2) * 8:(n - 1) * 8, 0:64],
                                    pn[:].rearrange("p (h e) -> p h e", e=64))
                        nc.vector.memset(vv3[:, :, 64:65], 1.0)
                        flush_ctx()
                        pending.append((ek, vv, tt))

                def qexp(c):
                    cs = slice(c * NCH, (c + 1) * NCH)
                    for m in range(DT):
                        qp = qp_pool.tile([P, NCH], F32, tag="q", name="q")
                        for t in range(DT // 2):
                            nc.tensor.matmul(
                                qp[:], wq_t[t][:, :, m * P:(m + 1) * P],
                                h_p[t][:, :, cs],
                                start=(t == 0), stop=(t == DT // 2 - 1),
                                perf_mode=DRM)
                        eq = eqp.tile([P, NCH], BF, tag=f"expq{c}_{m}",
                                      name=f"expq{c}_{m}")
                        nc.scalar.activation(eq[:], qp[:], AF.Exp)
                        expq_a[c][m] = eq

                _mark(nc, 'A:start')
                ln1_x_load(0)
                load_consts()
                load_w()
                ln1_stats(0)
                for c in range(NC):
                    # apply(c) first: it drains the single-buffered stats psum
                    # before stats(c+1) reuses it
                    ln1_apply(c)
                    if c + 1 < NC:
                        ln1_x_load(c + 1)
                        ln1_stats(c + 1)
                    kv_ctx(c)
                flush_ctx(last=True)

                _mark(nc, 'A:ctx_evict')
                ctx_sb = kvev.tile([P, 1024], F32, tag="ctxev", name="ctxev")
                nc.vector.tensor_copy(ctx_sb[:], ctx_ps[:])
                nc.sync.dma_start(ar_in[:], ctx_sb[:])

                # q-GEMM after the evict in program order: the scheduler
                # pulls it forward into main-loop bubbles, and whatever is
                # left covers the AllReduce latency window.
                for c in range(NC):
                    qexp(c)

            # AllGather + local add instead of AllReduce: half the wire, no
            # CCE reduce step on the collective path.
            nc.gpsimd.collective_compute(
                "AllGather", AluOpType.bypass, replica_groups=RG,
                ins=[ar_in[:].opt()], outs=[ar_out[:].opt()])
            ctxh0 = eqp.tile([P, 1024], F32, tag="ctxh0", name="ctxh0")
            ctxh1 = eqp.tile([P, 1024], F32, tag="ctxh1", name="ctxh1")
            nc.sync.dma_start(ctxh0[:], ar_out[0:P, :])
            nc.sync.dma_start(ctxh1[:], ar_out[P:2 * P, :])
            nc.vector.tensor_add(ctxg_sb[:], ctxh0[:], ctxh1[:])

            _mark(nc, 'A:q')
            # ---------- softmax tail: 1/sum + broadcast, overlaps AllReduce
            with tc.tile_pool(name="q_small", bufs=3) as qsm, \
                 tc.tile_pool(name="bc_psum", bufs=2, space="PSUM") as bc_pool, \
                 tc.tile_pool(name="ssum_psum", bufs=2, space="PSUM") as sp_pool:
                for c in range(NC):
                    cs = slice(c * NCH, (c + 1) * NCH)
                    expq = expq_a[c]
                    s_ps = sp_pool.tile([32, NCH], F32, tag="ssum", name="ssum")
                    for m in range(DT):
                        nc.tensor.matmul(s_ps[:], indsum_t[m][:], expq[m][:],
                                         start=(m == 0), stop=(m == DT - 1))
                    rs = qsm.tile([32, NCH], BF, tag="recS", name="recS")
                    nc.vector.tensor_copy(rs[:], s_ps[:])
                    with nc.allow_low_precision(reason="softmax scale in bf16"):
                        nc.vector.reciprocal(rs[0:H, :], s_ps[0:H, :])
                    for m in range(DT):
                        bc = bc_pool.tile([P, NCH], F32, tag="bc", name="bc")
                        nc.tensor.matmul(bc[:], indbc_t[m][:], rs[:],
                                         start=True, stop=True)
                        with nc.allow_low_precision(reason="softmaxed q in bf16"):
                            nc.vector.tensor_mul(qs_sb[m][:, cs], expq[m][:], bc[:])

        # phase-B weights, fp8, SBUF-resident (loaded once at B start; the
        # first consumers — wout of chunk 0 — need only the first 1 MB).
        bw_cm = tc.tile_pool(name="bw_hold", bufs=1)
        bw = bw_cm.__enter__()
        wout_t = [bw.tile([P, D], F8, tag=f"wo{m}", name=f"wo{m}")
                  for m in range(DT)]
        w1_t = [bw.tile([P, D], F8, tag=f"w1_{m}", name=f"w1_{m}")
                for m in range(DDT)]
        w2_t = [bw.tile([P, DD], F8, tag=f"w2_{m}", name=f"w2_{m}")
                for m in range(DDT)]
        w3_t = [bw.tile([P, DD], F8, tag=f"w3_{m}", name=f"w3_{m}")
                for m in range(DT)]
        for m in range(DT):
            nc.sync.dma_start(wout_t[m][:], tn["wout"][m * P:(m + 1) * P, :])
        for m in range(DDT):
            nc.sync.dma_start(w1_t[m][:], tn["w1"][m * P:(m + 1) * P, :])
        for m in range(DDT):
            nc.sync.dma_start(w2_t[m][:], tn["w2"][m * P:(m + 1) * P, :])
        for m in range(DT):
            nc.sync.dma_start(w3_t[m][:], tn["w3"][m * P:(m + 1) * P, :])

        _mark(nc, 'ctxnorm')
        # normalize ctx into block-diagonal head-pair lhsT tiles (bf16):
        # ctxd[:, 128i:128(i+1)] = [[ctx_{2i}*zr, 0], [0, ctx_{2i+1}*zr]]
        bhold_cm = tc.tile_pool(name="b_hold", bufs=1)
        bhold = bhold_cm.__enter__()
        ctxd_sb = bhold.tile([P, 1024], BF, tag="ctxd", name="ctxd")
        zr_sb = bhold.tile([P, 8], F32, tag="zr", name="zr")
        for i in range(8):
            c0 = _ctx_col(i)
            nc.vector.reciprocal(zr_sb[:, i:i + 1], ctxg_sb[:, c0 + 64:c0 + 65])
        nc.scalar.mul(zr_sb[:], zr_sb[:], DH ** -0.5)
        nc.vector.memset(ctxd_sb[:], 0.0)
        for h16 in range(H):
            i, j = h16 // 2, h16 % 2
            c0 = _ctx_col(i)
            nc.vector.tensor_scalar(
                ctxd_sb[64 * j:64 * j + 64, 128 * i + 64 * j:128 * i + 64 * j + 64],
                ctxg_sb[64 * j:64 * j + 64, c0:c0 + 64],
                zr_sb[64 * j:64 * j + 64, i:i + 1], None, AluOpType.mult)

        # =================================================================
        # PHASE B: per token chunk attn -> w_out+res -> LN2 -> MLP+res
        # (cross-chunk pipelined; MLP + w_out GEMMs in fp8 DoubleRow)
        # =================================================================
        with tc.tile_pool(name="b_attp", bufs=1) as bap, \
             tc.tile_pool(name="b_act2", bufs=2) as bact2, \
             tc.tile_pool(name="b_stream", bufs=4) as bstr, \
             tc.tile_pool(name="b_y", bufs=1) as by_pool, \
             tc.tile_pool(name="b_work", bufs=2) as bw2, \
             tc.tile_pool(name="b_lnw", bufs=1) as blnw, \
             tc.tile_pool(name="b_psum", bufs=2, space="PSUM") as bp, \
             tc.tile_pool(name="b_stat_psum", bufs=1, space="PSUM") as bsp:
            x2_c = {}
            h2_c = {}
            stats_c = {}

            def stage_a(n):
                cs = slice(n * NCH, (n + 1) * NCH)
                _mark(nc, f'B{n}:attn')
                att_p = [bap.tile([P, 2, NCH], F8, tag=f"attp{t}",
                                  name=f"attp{t}") for t in range(DT // 2)]
                for i in range(DT):
                    ap_ps = bp.tile([P, NCH], F32, tag="attn", name="attn")
                    nc.tensor.matmul(ap_ps[:], ctxd_sb[:, P * i:P * (i + 1)],
                                     qs_sb[i][:, cs], start=True, stop=True)
                    with nc.allow_low_precision(reason="attn out fp8 for DR gemm"):
                        nc.scalar.mul(att_p[i // 2][:, i % 2, :], ap_ps[:], 1.0)
                _mark(nc, f'B{n}:wout')
                x2_t = []
                mu2 = bsp.tile([P, NCH], F32, tag="mu2", name="mu2")
                ms2 = bsp.tile([P, NCH], F32, tag="ms2", name="ms2")
                for m in range(DT):
                    wo_ps = bp.tile([P, NCH], F32, tag="wout", name="wout")
                    w3d = wout_t[m][:].rearrange("p (k c) -> p k c", c=P)
                    for t in range(DT // 2):
                        nc.tensor.matmul(wo_ps[:], w3d[:, 2 * t:2 * t + 2, :],
                                         att_p[t][:, :, :],
                                         start=(t == 0), stop=(t == DT // 2 - 1),
                                         perf_mode=DRM)
                    xc = bw2.tile([P, NCH], FR, tag="xc", name="xc")
                    nc.sync.dma_start(xc[:], xT[m * P:(m + 1) * P, cs])
                    x2 = bact2.tile([P, NCH], FR, tag=f"x2_{m}", name=f"x2_{m}")
                    nc.vector.scalar_tensor_tensor(
                        x2[:], wo_ps[:], vecs["bout"][:, m:m + 1], xc[:],
                        AluOpType.add, AluOpType.add)
                    x2_t.append(x2)
                    sq = bw2.tile([P, NCH], FR, tag="sq2", name="sq2")
                    nc.scalar.activation(sq[:], x2[:], AF.Square)
                    nc.tensor.matmul(mu2[:], ones_t[:], x2[:],
                                     start=(m == 0), stop=(m == DT - 1))
                    nc.tensor.matmul(ms2[:], ones_t[:], sq[:],
                                     start=(m == 0), stop=(m == DT - 1))
                x2_c[n] = x2_t
                stats_c[n] = (mu2, ms2)

            def stage_ln(n):
                _mark(nc, f'B{n}:ln2')
                mu2, ms2 = stats_c.pop(n)
                rstd, murstd = ln_stats_to_scales(mu2, ms2, blnw, "2")
                h2p = [bact2.tile([P, 2, NCH], F8, tag=f"h2p{t}",
                                  name=f"h2p{t}") for t in range(DT // 2)]
                for m in range(DT):
                    tmp = bw2.tile([P, NCH], F32, tag="h2t", name="h2t")
                    nc.vector.tensor_mul(tmp[:], x2_c[n][m][:], rstd[:])
                    nc.vector.tensor_sub(tmp[:], tmp[:], murstd[:])
                    # h2 = tmp*g2 + b2ln, quantized to fp8 on the ACT engine
                    with nc.allow_low_precision(reason="h2 fp8 for DR gemm"):
                        nc.scalar.activation(
                            h2p[m // 2][:, m % 2, :], tmp[:], AF.Identity,
                            scale=vecs["lng2"][:, m:m + 1],
                            bias=vecs["lnb2"][:, m:m + 1])
                h2_c[n] = h2p

            def stage_mlp(n):
                cs = slice(n * NCH, (n + 1) * NCH)
                h2p = h2_c.pop(n)
                x2_t = x2_c.pop(n)
                _mark(nc, f'B{n}:y1')
                y1p = [by_pool.tile([P, 2, NCH], F8, tag=f"y1p{t}",
                                    name=f"y1p{t}") for t in range(DDT // 2)]
                for m in range(DDT):
                    y_ps = bp.tile([P, NCH], F32, tag="mlp", name="mlp")
                    w3d = w1_t[m][:].rearrange("p (k c) -> p k c", c=P)
                    for t in range(DT // 2):
                        nc.tensor.matmul(y_ps[:], w3d[:, 2 * t:2 * t + 2, :],
                                         h2p[t][:, :, :],
                                         start=(t == 0), stop=(t == DT // 2 - 1),
                                         perf_mode=DRM)
                    with nc.allow_low_precision(reason="y1 fp8 for DR gemm"):
                        nc.scalar.activation(y1p[m // 2][:, m % 2, :], y_ps[:],
                                             AF.Gelu, bias=vecs["b1"][:, m:m + 1])
                _mark(nc, f'B{n}:y2')
                y2p = [by_pool.tile([P, 2, NCH], F8, tag=f"y2p{t}",
                                    name=f"y2p{t}") for t in range(DDT // 2)]
                for m in range(DDT):
                    y_ps = bp.tile([P, NCH], F32, tag="mlp", name="mlp")
                    w3d = w2_t[m][:].rearrange("p (k c) -> p k c", c=P)
                    for t in range(DDT // 2):
                        nc.tensor.matmul(y_ps[:], w3d[:, 2 * t:2 * t + 2, :],
                                         y1p[t][:, :, :],
                                         start=(t == 0), stop=(t == DDT // 2 - 1),
                                         perf_mode=DRM)
                    with nc.allow_low_precision(reason="y2 fp8 for DR gemm"):
                        nc.scalar.activation(y2p[m // 2][:, m % 2, :], y_ps[:],
                                             AF.Gelu, bias=vecs["b2"][:, m:m + 1])
                _mark(nc, f'B{n}:y3')
                for m in range(DT):
                    y_ps = bp.tile([P, NCH], F32, tag="mlp", name="mlp")
                    w3d = w3_t[m][:].rearrange("p (k c) -> p k c", c=P)
                    for t in range(DDT // 2):
                        nc.tensor.matmul(y_ps[:], w3d[:, 2 * t:2 * t + 2, :],
                                         y2p[t][:, :, :],
                                         start=(t == 0), stop=(t == DDT // 2 - 1),
                                         perf_mode=DRM)
                    ot = bw2.tile([P, NCH], F32, tag="ot", name="ot")
                    nc.vector.scalar_tensor_tensor(
                        ot[:], y_ps[:], vecs["b3"][:, m:m + 1], x2_t[m][:],
                        AluOpType.add, AluOpType.add)
                    nc.sync.dma_start(out_d[m * P:(m + 1) * P, cs], ot[:])

            stage_a(0)
            stage_ln(0)
            for n in range(NC):
                if n + 1 < NC:
                    stage_a(n + 1)
                stage_mlp(n)
                if n + 1 < NC:
                    stage_ln(n + 1)
        bhold_cm.__exit__(None, None, None)
        bw_cm.__exit__(None, None, None)


# =========================================================================
# host side
# =========================================================================

def _sinusoidal_pe(seq_len, d_model):
    pos = np.arange(seq_len, dtype=np.float32)[:, None]
    div = np.exp(np.arange(0, d_model, 2, dtype=np.float32)
                 * (-np.log(10000.0) / d_model))
    pe = np.zeros((seq_len, d_model), np.float32)
    pe[:, 0::2] = np.sin(pos * div)
    pe[:, 1::2] = np.cos(pos * div)
    return pe


def _col_block(w):
    """[K, M] -> [M//128 * 128, K] tiles: cb[m*128+p, k*128+c] = w[k*128+p, m*128+c]."""
    K, M = w.shape
    kt, mt = K // P, M // P
    return np.ascontiguousarray(
        w.reshape(kt, P, mt, P).transpose(2, 1, 0, 3).reshape(mt * P, kt * P))


def _vec_tiles(v, ntiles):
    return np.ascontiguousarray(np.asarray(v, np.float32).reshape(ntiles, P).T)


def make_in_maps(inputs, S):
    T = B * S // NCORES
    x = np.asarray(inputs["x"], np.float32)
    # peb = (pe + ln1_b) / ln1_g — the LN1 gain is applied after the pe-add
    # via the ACT copy's scale slot (h = (LN + peb) * g)
    g1 = np.asarray(inputs["ln1_g"], np.float32)
    pe = (_sinusoidal_pe(S, D) + np.asarray(inputs["ln1_b"], np.float32)[None, :]) / g1[None, :]

    indsum = np.zeros((DT * P, 32), np.float32)
    indbc = np.zeros((DT * 32, P), np.float32)
    for t in range(DT):
        for j in range(P):
            h = 2 * t + (1 if j >= 64 else 0)
            indsum[t * P + j, h] = 1.0
            indbc[t * 32 + h, j] = 1.0

    F8NP = ml_dtypes.float8_e4m3
    BFNP = ml_dtypes.bfloat16

    def _pair_pack(w):
        """[K, M] -> [K//2, 2M]: row 128t+p, col j*M+c = w[(2t+j)*128+p, c]."""
        K, M = w.shape
        return np.ascontiguousarray(
            w.reshape(K // 256, 2, P, M).transpose(0, 2, 1, 3).reshape(K // 2, 2 * M))

    wqkv = np.asarray(inputs["w_qkv"], np.float32)
    shared = {
        "wq": _pair_pack(np.ascontiguousarray(wqkv[:, :D])).astype(F8NP),
        "wkv": _pair_pack(np.ascontiguousarray(wqkv[:, D:])).astype(F8NP),
        "wout": _col_block(np.asarray(inputs["w_out"], np.float32)).astype(F8NP),
        "w1": _col_block(np.asarray(inputs["w1"], np.float32)).astype(F8NP),
        "w2": _col_block(np.asarray(inputs["w2"], np.float32)).astype(F8NP),
        "w3": _col_block(np.asarray(inputs["w3"], np.float32)).astype(F8NP),
        "lng1": _vec_tiles(inputs["ln1_g"], DT),
        "lng2": _vec_tiles(inputs["ln2_g"], DT),
        "lnb2": _vec_tiles(inputs["ln2_b"], DT),
        "bout": _vec_tiles(inputs["b_out"], DT),
        "b1": _vec_tiles(inputs["b1"], DDT),
        "b2": _vec_tiles(inputs["b2"], DDT),
        "b3": _vec_tiles(inputs["b3"], DT),
        "ones": np.ones((P, P), np.float32),
        "indsum": indsum.astype(BFNP),
        "indbc": indbc.astype(BFNP),
    }
    in_maps = []
    for c in range(NCORES):
        b, hhalf = divmod(c, NCORES // B)
        s0 = hhalf * T
        m = dict(shared)
        m["xT"] = np.ascontiguousarray(x[b, s0:s0 + T, :].T)
        m["peb"] = np.ascontiguousarray(pe[s0:s0 + T, :].T).astype(BFNP)
        in_maps.append(m)
    return in_maps


def gather(results, S):
    T = B * S // NCORES
    full = np.empty((B, S, D), np.float32)
    for c in range(NCORES):
        b, hhalf = divmod(c, NCORES // B)
        s0 = hhalf * T
        full[b, s0:s0 + T, :] = results[c]["out"].T
    return full


_GRAPH_CACHE = {}


def _get_graph(S):
    T = B * S // NCORES
    if T not in _GRAPH_CACHE:
        _GRAPH_CACHE[T] = build_graph(T)
    return _GRAPH_CACHE[T]


def run(inputs, S, **kw):
    nc = _get_graph(S)
    in_maps = make_in_maps(inputs, S)
    res = run_bass_kernel_spmd(nc, in_maps, core_ids=list(range(NCORES)), **kw)
    return gather(res.results, S), res


def kernel(**inputs):
    out, _ = run(inputs, S_FULL)
    return out


# revision 48
# speedup vs baseline: 1.2559x; 1.0084x over previous
"""Trainium2 Bass kernel for nn_Attn_86784109183632.

Transformer block: LN1 -> +sinusoidal PE -> linear (efficient) attention ->
w_out + residual -> LN2 -> 3-layer gelu MLP + residual.
B=4, S=4096, D=1024, H=16, dh=64.

Sharding: data-parallel over (batch, seq-half) -> 8 cores x 2048 tokens.
The only cross-core term is the k-softmax normalizer and k^T v context
(sums over the sequence axis), reduced with a tiny pairwise AllReduce
([128,1024] fp32) between the two cores holding the same batch, overlapped
with the q projection + q softmax.

Precision plan (tolerance is rel-max 2e-2; measured ~1.3e-2 in numpy sim):
  - x / residuals / LN statistics: fp32(r)
  - attention path (h, w_qkv, exp(k), v, softmaxed q, ctx): bf16
  - MLP + w_out GEMMs: fp8 e4m3 with DoubleRow perf mode (2 k-blocks per
    matmul pass), weights SBUF-resident, activations quantized on the fly
    by the Activation engine into paired [128, 2, 512] tiles.
Softmaxed q stays SBUF-resident between the q phase and phase B (no DRAM
spill). All activations are dim-major [dims, tokens].
"""

import sys

if "/opt/trn_rl_repo" not in sys.path:
    sys.path.insert(0, "/opt/trn_rl_repo")

import ml_dtypes
import numpy as np

import concourse.mybir as mybir
import concourse.tile as tile
from concourse import bacc
from concourse.alu_op_type import AluOpType
from concourse.bass_utils import run_bass_kernel_spmd

P = 128
D = 1024
DD = 2048  # mlp hidden
H = 16
DH = 64
B = 4
S_FULL = 4096
NCORES = 8
EPS = 1e-6

FR = mybir.dt.float32r
F32 = mybir.dt.float32
BF = mybir.dt.bfloat16
F8 = mybir.dt.float8e4
AF = mybir.ActivationFunctionType
DRM = mybir.MatmulPerfMode.DoubleRow

DT = D // P        # 8 d-tiles
DDT = DD // P      # 16 mlp-tiles
NCH = 512          # token chunk (one fp32 psum bank)


def _ctx_col(i):
    """Free-dim offset of head-pair block i inside ctx psum (4 pairs/bank)."""
    return 512 * (i // 4) + 65 * (i % 4)


def build_graph(T):
    """Build the SPMD graph for T tokens per core. T % 512 == 0."""
    assert T % NCH == 0
    TT = T // P           # token tiles
    NC = T // NCH         # token chunks

    nc = bacc.Bacc("TRN2", target_bir_lowering=False, debug=False,
                   num_devices=NCORES)

    tn = {}
    tn["xT"] = nc.dram_tensor("xT", [D, T], FR, kind="ExternalInput")
    tn["peb"] = nc.dram_tensor("peb", [D, T], BF, kind="ExternalInput")
    # q/kv weights pair-packed for DoubleRow: row 128t+p, col j*M+c holds
    # w[(2t+j)*128+p, c]
    tn["wq"] = nc.dram_tensor("wq", [D // 2, 2 * D], F8, kind="ExternalInput")
    tn["wkv"] = nc.dram_tensor("wkv", [D // 2, 4 * D], F8, kind="ExternalInput")
    tn["wout"] = nc.dram_tensor("wout", [D, D], F8, kind="ExternalInput")  # col-block
    tn["w1"] = nc.dram_tensor("w1", [DD, D], F8, kind="ExternalInput")  # col-block
    tn["w2"] = nc.dram_tensor("w2", [DD, DD], F8, kind="ExternalInput")  # col-block
    tn["w3"] = nc.dram_tensor("w3", [D, DD], F8, kind="ExternalInput")  # col-block
    # per-dim vectors laid out [128, n_tiles] (column t = dims 128t..128t+127)
    for name, nt in [("lng1", DT), ("lng2", DT), ("lnb2", DT),
                     ("bout", DT), ("b1", DDT), ("b2", DDT), ("b3", DT)]:
        tn[name] = nc.dram_tensor(name, [P, nt], F32, kind="ExternalInput")
    tn["ones"] = nc.dram_tensor("ones", [P, P], FR, kind="ExternalInput")
    tn["indsum"] = nc.dram_tensor("indsum", [DT * P, 32], BF, kind="ExternalInput")
    tn["indbc"] = nc.dram_tensor("indbc", [DT * 32, P], BF, kind="ExternalInput")
    tn["out"] = nc.dram_tensor("out", [D, T], F32, kind="ExternalOutput")

    PHASE_MARKS.clear()
    with tile.TileContext(nc) as tc:
        _build_body(nc, tc, T, TT, NC, tn)
    import json as _json
    _json.dump(PHASE_MARKS, open("/tmp/phase_marks.json", "w"))
    nc.compile()
    return nc


PHASE_MARKS = []


def _mark(nc, label):
    PHASE_MARKS.append((label, nc.next_id()))


def _build_body(nc, tc, T, TT, NC, tn):
    xT, out_d = tn["xT"], tn["out"]
    RG = [[0, 1], [2, 3], [4, 5], [6, 7]]

    with tc.tile_pool(name="const", bufs=1) as const, \
         tc.tile_pool(name="ctx_hold", bufs=1) as ctx_hold, \
         tc.tile_pool(name="qs_hold", bufs=1) as qs_hold, \
         tc.tile_pool(name="dram", bufs=1, space="DRAM") as dram:

        # ------------- constants (tiles now, DMAs deferred so the first
        # x chunk wins the DMA queue) -------------
        ones_t = const.tile([P, P], FR)
        indsum_t, indbc_t = [], []
        for t in range(DT):
            it = const.tile([P, 32], BF, tag=f"indsum{t}", name=f"indsum{t}")
            indsum_t.append(it)
            bt = const.tile([32, P], BF, tag=f"indbc{t}", name=f"indbc{t}")
            indbc_t.append(bt)
        vecs = {}
        for name, nt in [("lng1", DT), ("lng2", DT), ("lnb2", DT),
                         ("bout", DT), ("b1", DDT), ("b2", DDT), ("b3", DT)]:
            vecs[name] = const.tile([P, nt], F32, tag=name, name=name)

        def load_consts():
            nc.sync.dma_start(ones_t[:], tn["ones"][:])
            for t in range(DT):
                nc.sync.dma_start(indsum_t[t][:], tn["indsum"][t * P:(t + 1) * P, :])
                nc.sync.dma_start(indbc_t[t][:], tn["indbc"][t * 32:(t + 1) * 32, :])
            for name in vecs:
                nc.sync.dma_start(vecs[name][:], tn[name][:])

        eps_t = const.tile([P, 1], F32, tag="eps", name="eps")
        nc.vector.memset(eps_t[:], EPS)
        ctxg_sb = ctx_hold.tile([P, 1024], F32)   # ctx after AllReduce

        # softmaxed q, SBUF-resident through phase B
        qs_sb = [qs_hold.tile([P, T], BF, tag=f"qs{m}", name=f"qs{m}")
                 for m in range(DT)]

        ar_in = dram.tile([P, 1024], F32, tag="ar_in", name="ar_in")
        ar_out = dram.tile([2 * P, 1024], F32, tag="ar_out", name="ar_out")

        def ln_stats_to_scales(mu_ps, ms_ps, pool, tagsfx, tmp_pool=None, w=NCH):
            """mu_ps/ms_ps: psum [P, >=w] broadcast sums of x and x^2 over D.
            Returns (rstd_b, murstd_b) SBUF [P, NCH] f32 (first w cols valid)."""
            tpool = tmp_pool if tmp_pool is not None else pool
            s = slice(0, w)
            mu_n = tpool.tile([P, NCH], F32, tag="t_mun" + tagsfx)
            nc.scalar.mul(mu_n[:, s], mu_ps[:, s], 1.0 / D)
            var = tpool.tile([P, NCH], F32, tag="t_var" + tagsfx)
            nc.scalar.activation(var[:, s], mu_ps[:, s], AF.Square, scale=1.0 / D)
            ex2 = tpool.tile([P, NCH], F32, tag="t_ex2" + tagsfx)
            nc.scalar.mul(ex2[:, s], ms_ps[:, s], 1.0 / D)
            nc.vector.tensor_sub(var[:, s], ex2[:, s], var[:, s])
            sd = tpool.tile([P, NCH], F32, tag="t_sd" + tagsfx)
            nc.scalar.activation(sd[:, s], var[:, s], AF.Sqrt, bias=eps_t[:])
            rb = pool.tile([P, NCH], F32, tag="rb" + tagsfx)
            nc.vector.reciprocal(rb[:, s], sd[:, s])
            mb = pool.tile([P, NCH], F32, tag="mb" + tagsfx)
            nc.vector.tensor_mul(mb[:, s], mu_n[:, s], rb[:, s])
            return rb, mb

        # =================================================================
        # PHASE A
        # =================================================================
        # phase-A chunking: first 512-token chunk split in two so the LN1
        # stats/apply serial chain gates less work at startup
        CHUNKS = [(0, 256), (256, 256)] + [(c * NCH, NCH) for c in range(1, NC)]

        with tc.tile_pool(name="h_pool", bufs=1) as h_pool, \
             tc.tile_pool(name="expq_pool", bufs=1) as eqp:
            # h as fp8 k-pair tiles for DoubleRow: h_p[t][:, j, s] = h block 2t+j
            h_p = [h_pool.tile([P, 2, T], F8, tag=f"h{t}", name=f"h{t}")
                   for t in range(DT // 2)]
            expq_a = [[None] * DT for _ in range(len(CHUNKS))]

            # ---- fused: LN1 + kv-GEMM + ctx + q-GEMM, chunk-pipelined ----
            with tc.tile_pool(name="wkv_pool", bufs=1) as wkv_pool, \
                 tc.tile_pool(name="ln1_work", bufs=2) as lnw, \
                 tc.tile_pool(name="ln1_tmp", bufs=1) as lntmp, \
                 tc.tile_pool(name="ln1_x", bufs=2) as lnx, \
                 tc.tile_pool(name="ln1_stream", bufs=2) as lns, \
                 tc.tile_pool(name="kv_work", bufs=2) as kvw, \
                 tc.tile_pool(name="kv_ev", bufs=1) as kvev, \
                 tc.tile_pool(name="ln1_psum", bufs=1, space="PSUM") as lnp, \
                 tc.tile_pool(name="mm_psum", bufs=2, space="PSUM") as kvp_pool, \
                 tc.tile_pool(name="q_psum", bufs=2, space="PSUM") as qp_pool, \
                 tc.tile_pool(name="ctx_psum", bufs=1, space="PSUM") as ctxp_pool:
                ctx_ps = ctxp_pool.tile([P, 1024], F32, tag="ctx", name="ctx")
                wkv_t, wq_t = [], []

                def load_w():
                    for t in range(DT // 2):
                        wt = wkv_pool.tile([P, 2, 2 * D], F8, tag=f"wkv{t}",
                                           name=f"wkv{t}")
                        nc.sync.dma_start(
                            wt[:].rearrange("p j c -> p (j c)"),
                            tn["wkv"][t * P:(t + 1) * P, :])
                        wkv_t.append(wt)
                        qt = wkv_pool.tile([P, 2, D], F8, tag=f"wq{t}",
                                           name=f"wq{t}")
                        nc.sync.dma_start(
                            qt[:].rearrange("p j c -> p (j c)"),
                            tn["wq"][t * P:(t + 1) * P, :])
                        wq_t.append(qt)

                stats_ps = {}
                xcur = {}

                def ln1_x_load(c0, w):
                    cs = slice(c0, c0 + w)
                    xs = []
                    for k in range(DT):
                        xk = lnx.tile([P, NCH], FR, tag=f"xc{k}", name=f"xc{k}")
                        nc.sync.dma_start(xk[:, 0:w], xT[k * P:(k + 1) * P, cs])
                        xs.append(xk)
                    xcur[c0] = xs

                def ln1_stats(c0, w):
                    mu = lnp.tile([P, NCH], F32, tag="mu", name="mu")
                    ms = lnp.tile([P, NCH], F32, tag="ms", name="ms")
                    for k in range(DT):
                        xk = xcur[c0][k]
                        sq = lns.tile([P, NCH], FR, tag="sq", name="sq")
                        nc.scalar.activation(sq[:, 0:w], xk[:, 0:w], AF.Square)
                        nc.tensor.matmul(mu[:, 0:w], ones_t[:], xk[:, 0:w],
                                         start=(k == 0), stop=(k == DT - 1))
                        nc.tensor.matmul(ms[:, 0:w], ones_t[:], sq[:, 0:w],
                                         start=(k == 0), stop=(k == DT - 1))
                    stats_ps[c0] = (mu, ms)

                def ln1_apply(c0, w):
                    cs = slice(c0, c0 + w)
                    mu, ms = stats_ps.pop(c0)
                    rb, mb = ln_stats_to_scales(mu, ms, lnw, "1", lntmp, w=w)
                    xs = xcur.pop(c0)
                    for k in range(DT):
                        pk = lns.tile([P, NCH], BF, tag="pe", name="pe")
                        nc.sync.dma_start(pk[:, 0:w], tn["peb"][k * P:(k + 1) * P, cs])
                        tmp = lns.tile([P, NCH], F32, tag="lnt", name="lnt")
                        nc.vector.tensor_mul(tmp[:, 0:w], xs[k][:, 0:w], rb[:, 0:w])
                        nc.vector.tensor_sub(tmp[:, 0:w], tmp[:, 0:w], mb[:, 0:w])
                        # h = (LN1(x) + (pe + b)/g) * g — gain applied via the
                        # ACT copy's scale slot; host sends peb = (pe + b)/g
                        nc.vector.tensor_add(tmp[:, 0:w], tmp[:, 0:w], pk[:, 0:w])
                        with nc.allow_low_precision(reason="h fp8 for DR gemms"):
                            nc.scalar.activation(h_p[k // 2][:, k % 2, cs],
                                                 tmp[:, 0:w], AF.Identity,
                                                 scale=vecs["lng1"][:, k:k + 1])

                pending = []  # (ek, vv, global_tt) awaiting ctx matmuls

                def flush_ctx(last=False):
                    while pending:
                        ek, vv, pt = pending.pop(0)
                        for h16 in range(H):
                            i, j = h16 // 2, h16 % 2
                            c0 = _ctx_col(i)
                            nc.tensor.matmul(
                                ctx_ps[64 * j:64 * j + 64, c0:c0 + 65],
                                ek[:, 64 * h16:64 * h16 + 64],
                                vv[:, h16 * 65:(h16 + 1) * 65],
                                start=(pt == 0 and h16 in (0, 1, 8, 9)),
                                stop=(pt == TT - 1 and h16 in (6, 7, 14, 15)))

                def kv_ctx(c0, w):
                    for lt in range(w // P):
                        tt = c0 // P + lt
                        ts_ = slice(tt * P, (tt + 1) * P)
                        ek = kvw.tile([P, D], BF, tag="ek", name="ek")
                        vv = kvw.tile([P, H * 65], BF, tag="vv", name="vv")
                        vv3 = vv[:].rearrange("p (h e) -> p h e", e=65)
                        for n in range(4):
                            pn = kvp_pool.tile([P, 512], F32, tag="kv", name="kv")
                            for t in range(DT // 2):
                                nc.tensor.matmul(
                                    pn[:], h_p[t][:, :, ts_],
                                    wkv_t[t][:, :, n * 512:(n + 1) * 512],
                                    start=(t == 0), stop=(t == DT // 2 - 1),
                                    perf_mode=DRM)
                            if n < 2:
                                nc.scalar.activation(ek[:, n * 512:(n + 1) * 512],
                                                     pn[:], AF.Exp)
                            else:
                                nc.vector.tensor_copy(
                                    vv3[:, (n - 2) * 8:(n - 1) * 8, 0:64],
                                    pn[:].rearrange("p (h e) -> p h e", e=64))
                        nc.vector.memset(vv3[:, :, 64:65], 1.0)
                        flush_ctx()
                        pending.append((ek, vv, tt))

                def qexp(ci, c0, w):
                    cs = slice(c0, c0 + w)
                    for m in range(DT):
                        qp = qp_pool.tile([P, NCH], F32, tag="q", name="q")
                        for t in range(DT // 2):
                            nc.tensor.matmul(
                                qp[:, 0:w], wq_t[t][:, :, m * P:(m + 1) * P],
                                h_p[t][:, :, cs],
                                start=(t == 0), stop=(t == DT // 2 - 1),
                                perf_mode=DRM)
                        eq = eqp.tile([P, w], BF, tag=f"expq{ci}_{m}",
                                      name=f"expq{ci}_{m}")
                        nc.scalar.activation(eq[:, 0:w], qp[:, 0:w], AF.Exp)
                        expq_a[ci][m] = eq

                _mark(nc, 'A:start')
                # first chunk split in two to shorten the pipeline fill
                ln1_x_load(0, 256)
                load_consts()
                load_w()
                ln1_stats(0, 256)
                for i, (c0, w) in enumerate(CHUNKS):
                    # apply first: it drains the single-buffered stats psum
                    # before the next stats reuses it
                    ln1_apply(c0, w)
                    if i + 1 < len(CHUNKS):
                        n0, nw = CHUNKS[i + 1]
                        ln1_x_load(n0, nw)
                        ln1_stats(n0, nw)
                    kv_ctx(c0, w)
                flush_ctx(last=True)

                _mark(nc, 'A:ctx_evict')
                ctx_sb = kvev.tile([P, 1024], F32, tag="ctxev", name="ctxev")
                nc.vector.tensor_copy(ctx_sb[:], ctx_ps[:])
                nc.sync.dma_start(ar_in[:], ctx_sb[:])

                # q-GEMM after the evict in program order: the scheduler
                # pulls it forward into main-loop bubbles, and whatever is
                # left covers the AllReduce latency window.
                for ci, (c0, w) in enumerate(CHUNKS):
                    qexp(ci, c0, w)

            # AllGather + local add instead of AllReduce: half the wire, no
            # CCE reduce step on the collective path.
            nc.gpsimd.collective_compute(
                "AllGather", AluOpType.bypass, replica_groups=RG,
                ins=[ar_in[:].opt()], outs=[ar_out[:].opt()])
            ctxh0 = eqp.tile([P, 1024], F32, tag="ctxh0", name="ctxh0")
            ctxh1 = eqp.tile([P, 1024], F32, tag="ctxh1", name="ctxh1")
            nc.sync.dma_start(ctxh0[:], ar_out[0:P, :])
            nc.sync.dma_start(ctxh1[:], ar_out[P:2 * P, :])
            nc.vector.tensor_add(ctxg_sb[:], ctxh0[:], ctxh1[:])

            _mark(nc, 'A:q')
            # ---------- softmax tail: 1/sum + broadcast, overlaps AllReduce
            with tc.tile_pool(name="q_small", bufs=3) as qsm, \
                 tc.tile_pool(name="bc_psum", bufs=2, space="PSUM") as bc_pool, \
                 tc.tile_pool(name="ssum_psum", bufs=2, space="PSUM") as sp_pool:
                for ci, (c0, w) in enumerate(CHUNKS):
                    cs = slice(c0, c0 + w)
                    expq = expq_a[ci]
                    s_ps = sp_pool.tile([32, NCH], F32, tag="ssum", name="ssum")
                    for m in range(DT):
                        nc.tensor.matmul(s_ps[:, 0:w], indsum_t[m][:], expq[m][:],
                                         start=(m == 0), stop=(m == DT - 1))
                    rs = qsm.tile([32, NCH], BF, tag="recS", name="recS")
                    nc.vector.tensor_copy(rs[:, 0:w], s_ps[:, 0:w])
                    with nc.allow_low_precision(reason="softmax scale in bf16"):
                        nc.vector.reciprocal(rs[0:H, 0:w], s_ps[0:H, 0:w])
                    for m in range(DT):
                        bc = bc_pool.tile([P, NCH], F32, tag="bc", name="bc")
                        nc.tensor.matmul(bc[:, 0:w], indbc_t[m][:], rs[:, 0:w],
                                         start=True, stop=True)
                        with nc.allow_low_precision(reason="softmaxed q in bf16"):
                            nc.vector.tensor_mul(qs_sb[m][:, cs], expq[m][:],
                                                 bc[:, 0:w])

        # phase-B weights, fp8, SBUF-resident (loaded once at B start; the
        # first consumers — wout of chunk 0 — need only the first 1 MB).
        bw_cm = tc.tile_pool(name="bw_hold", bufs=1)
        bw = bw_cm.__enter__()
        wout_t = [bw.tile([P, D], F8, tag=f"wo{m}", name=f"wo{m}")
                  for m in range(DT)]
        w1_t = [bw.tile([P, D], F8, tag=f"w1_{m}", name=f"w1_{m}")
                for m in range(DDT)]
        w2_t = [bw.tile([P, DD], F8, tag=f"w2_{m}", name=f"w2_{m}")
                for m in range(DDT)]
        w3_t = [bw.tile([P, DD], F8, tag=f"w3_{m}", name=f"w3_{m}")
                for m in range(DT)]
        for m in range(DT):
            nc.sync.dma_start(wout_t[m][:], tn["wout"][m * P:(m + 1) * P, :])
        for m in range(DDT):
            nc.sync.dma_start(w1_t[m][:], tn["w1"][m * P:(m + 1) * P, :])
        for m in range(DDT):
            nc.sync.dma_start(w2_t[m][:], tn["w2"][m * P:(m + 1) * P, :])
        for m in range(DT):
            nc.sync.dma_start(w3_t[m][:], tn["w3"][m * P:(m + 1) * P, :])

        _mark(nc, 'ctxnorm')
        # normalize ctx into block-diagonal head-pair lhsT tiles (bf16):
        # ctxd[:, 128i:128(i+1)] = [[ctx_{2i}*zr, 0], [0, ctx_{2i+1}*zr]]
        bhold_cm = tc.tile_pool(name="b_hold", bufs=1)
        bhold = bhold_cm.__enter__()
        ctxd_sb = bhold.tile([P, 1024], BF, tag="ctxd", name="ctxd")
        zr_sb = bhold.tile([P, 8], F32, tag="zr", name="zr")
        for i in range(8):
            c0 = _ctx_col(i)
            nc.vector.reciprocal(zr_sb[:, i:i + 1], ctxg_sb[:, c0 + 64:c0 + 65])
        nc.scalar.mul(zr_sb[:], zr_sb[:], DH ** -0.5)
        nc.vector.memset(ctxd_sb[:], 0.0)
        for h16 in range(H):
            i, j = h16 // 2, h16 % 2
            c0 = _ctx_col(i)
            nc.vector.tensor_scalar(
                ctxd_sb[64 * j:64 * j + 64, 128 * i + 64 * j:128 * i + 64 * j + 64],
                ctxg_sb[64 * j:64 * j + 64, c0:c0 + 64],
                zr_sb[64 * j:64 * j + 64, i:i + 1], None, AluOpType.mult)

        # =================================================================
        # PHASE B: per token chunk attn -> w_out+res -> LN2 -> MLP+res
        # (cross-chunk pipelined; MLP + w_out GEMMs in fp8 DoubleRow)
        # =================================================================
        with tc.tile_pool(name="b_attp", bufs=1) as bap, \
             tc.tile_pool(name="b_act2", bufs=2) as bact2, \
             tc.tile_pool(name="b_stream", bufs=4) as bstr, \
             tc.tile_pool(name="b_y", bufs=1) as by_pool, \
             tc.tile_pool(name="b_work", bufs=2) as bw2, \
             tc.tile_pool(name="b_lnw", bufs=1) as blnw, \
             tc.tile_pool(name="b_psum", bufs=2, space="PSUM") as bp, \
             tc.tile_pool(name="b_mlp_psum", bufs=4, space="PSUM") as bmp, \
             tc.tile_pool(name="b_stat_psum", bufs=1, space="PSUM") as bsp:
            x2_c = {}
            h2_c = {}
            stats_c = {}

            def stage_a(n):
                cs = slice(n * NCH, (n + 1) * NCH)
                _mark(nc, f'B{n}:attn')
                att_p = [bap.tile([P, 2, NCH], F8, tag=f"attp{t}",
                                  name=f"attp{t}") for t in range(DT // 2)]
                for i in range(DT):
                    ap_ps = bp.tile([P, NCH], F32, tag="awout", name="attn")
                    nc.tensor.matmul(ap_ps[:], ctxd_sb[:, P * i:P * (i + 1)],
                                     qs_sb[i][:, cs], start=True, stop=True)
                    with nc.allow_low_precision(reason="attn out fp8 for DR gemm"):
                        nc.scalar.mul(att_p[i // 2][:, i % 2, :], ap_ps[:], 1.0)
                _mark(nc, f'B{n}:wout')
                x2_t = []
                mu2 = bsp.tile([P, NCH], F32, tag="mu2", name="mu2")
                ms2 = bsp.tile([P, NCH], F32, tag="ms2", name="ms2")
                for m in range(DT):
                    wo_ps = bp.tile([P, NCH], F32, tag="awout", name="wout")
                    w3d = wout_t[m][:].rearrange("p (k c) -> p k c", c=P)
                    for t in range(DT // 2):
                        nc.tensor.matmul(wo_ps[:], w3d[:, 2 * t:2 * t + 2, :],
                                         att_p[t][:, :, :],
                                         start=(t == 0), stop=(t == DT // 2 - 1),
                                         perf_mode=DRM)
                    xc = bw2.tile([P, NCH], FR, tag="xc", name="xc")
                    nc.sync.dma_start(xc[:], xT[m * P:(m + 1) * P, cs])
                    x2 = bact2.tile([P, NCH], FR, tag=f"x2_{m}", name=f"x2_{m}")
                    nc.vector.scalar_tensor_tensor(
                        x2[:], wo_ps[:], vecs["bout"][:, m:m + 1], xc[:],
                        AluOpType.add, AluOpType.add)
                    x2_t.append(x2)
                    sq = bw2.tile([P, NCH], FR, tag="sq2", name="sq2")
                    nc.scalar.activation(sq[:], x2[:], AF.Square)
                    nc.tensor.matmul(mu2[:], ones_t[:], x2[:],
                                     start=(m == 0), stop=(m == DT - 1))
                    nc.tensor.matmul(ms2[:], ones_t[:], sq[:],
                                     start=(m == 0), stop=(m == DT - 1))
                x2_c[n] = x2_t
                stats_c[n] = (mu2, ms2)

            def stage_ln(n):
                _mark(nc, f'B{n}:ln2')
                mu2, ms2 = stats_c.pop(n)
                rstd, murstd = ln_stats_to_scales(mu2, ms2, blnw, "2")
                h2p = [bact2.tile([P, 2, NCH], F8, tag=f"h2p{t}",
                                  name=f"h2p{t}") for t in range(DT // 2)]
                for m in range(DT):
                    tmp = bw2.tile([P, NCH], F32, tag="h2t", name="h2t")
                    nc.vector.tensor_mul(tmp[:], x2_c[n][m][:], rstd[:])
                    nc.vector.tensor_sub(tmp[:], tmp[:], murstd[:])
                    # h2 = tmp*g2 + b2ln, quantized to fp8 on the ACT engine
                    with nc.allow_low_precision(reason="h2 fp8 for DR gemm"):
                        nc.scalar.activation(
                            h2p[m // 2][:, m % 2, :], tmp[:], AF.Identity,
                            scale=vecs["lng2"][:, m:m + 1],
                            bias=vecs["lnb2"][:, m:m + 1])
                h2_c[n] = h2p

            def stage_mlp(n):
                cs = slice(n * NCH, (n + 1) * NCH)
                h2p = h2_c.pop(n)
                x2_t = x2_c.pop(n)
                _mark(nc, f'B{n}:y1')
                y1p = [by_pool.tile([P, 2, NCH], F8, tag=f"y1p{t}",
                                    name=f"y1p{t}") for t in range(DDT // 2)]
                for m in range(DDT):
                    y_ps = bmp.tile([P, NCH], F32, tag="mlp", name="mlp")
                    w3d = w1_t[m][:].rearrange("p (k c) -> p k c", c=P)
                    for t in range(DT // 2):
                        nc.tensor.matmul(y_ps[:], w3d[:, 2 * t:2 * t + 2, :],
                                         h2p[t][:, :, :],
                                         start=(t == 0), stop=(t == DT // 2 - 1),
                                         perf_mode=DRM)
                    with nc.allow_low_precision(reason="y1 fp8 for DR gemm"):
                        nc.scalar.activation(y1p[m // 2][:, m % 2, :], y_ps[:],
                                             AF.Gelu, bias=vecs["b1"][:, m:m + 1])
                _mark(nc, f'B{n}:y2')
                y2p = [by_pool.tile([P, 2, NCH], F8, tag=f"y2p{t}",
                                    name=f"y2p{t}") for t in range(DDT // 2)]
                for m in range(DDT):
                    y_ps = bmp.tile([P, NCH], F32, tag="mlp", name="mlp")
                    w3d = w2_t[m][:].rearrange("p (k c) -> p k c", c=P)
                    for t in range(DDT // 2):
                        nc.tensor.matmul(y_ps[:], w3d[:, 2 * t:2 * t + 2, :],
                                         y1p[t][:, :, :],
                                         start=(t == 0), stop=(t == DDT // 2 - 1),
                                         perf_mode=DRM)
                    with nc.allow_low_precision(reason="y2 fp8 for DR gemm"):
                        nc.scalar.activation(y2p[m // 2][:, m % 2, :], y_ps[:],
                                             AF.Gelu, bias=vecs["b2"][:, m:m + 1])
                _mark(nc, f'B{n}:y3')
                for m in range(DT):
                    y_ps = bmp.tile([P, NCH], F32, tag="mlp", name="mlp")
                    w3d = w3_t[m][:].rearrange("p (k c) -> p k c", c=P)
                    for t in range(DDT // 2):
                        nc.tensor.matmul(y_ps[:], w3d[:, 2 * t:2 * t + 2, :],
                                         y2p[t][:, :, :],
                                         start=(t == 0), stop=(t == DDT // 2 - 1),
                                         perf_mode=DRM)
                    ot = bw2.tile([P, NCH], F32, tag="ot", name="ot")
                    nc.vector.scalar_tensor_tensor(
                        ot[:], y_ps[:], vecs["b3"][:, m:m + 1], x2_t[m][:],
                        AluOpType.add, AluOpType.add)
                    nc.sync.dma_start(out_d[m * P:(m + 1) * P, cs], ot[:])

            stage_a(0)
            stage_ln(0)
            for n in range(NC):
                if n + 1 < NC:
                    stage_a(n + 1)
                stage_mlp(n)
                if n + 1 < NC:
                    stage_ln(n + 1)
        bhold_cm.__exit__(None, None, None)
        bw_cm.__exit__(None, None, None)


# =========================================================================
# host side
# =========================================================================

def _sinusoidal_pe(seq_len, d_model):
    pos = np.arange(seq_len, dtype=np.float32)[:, None]
    div = np.exp(np.arange(0, d_model, 2, dtype=np.float32)
                 * (-np.log(10000.0) / d_model))
    pe = np.zeros((seq_len, d_model), np.float32)
    pe[:, 0::2] = np.sin(pos * div)
    pe[:, 1::2] = np.cos(pos * div)
    return pe


def _col_block(w):
    """[K, M] -> [M//128 * 128, K] tiles: cb[m*128+p, k*128+c] = w[k*128+p, m*128+c]."""
    K, M = w.shape
    kt, mt = K // P, M // P
    return np.ascontiguousarray(
        w.reshape(kt, P, mt, P).transpose(2, 1, 0, 3).reshape(mt * P, kt * P))


def _vec_tiles(v, ntiles):
    return np.ascontiguousarray(np.asarray(v, np.float32).reshape(ntiles, P).T)


def make_in_maps(inputs, S):
    T = B * S // NCORES
    x = np.asarray(inputs["x"], np.float32)
    # peb = (pe + ln1_b) / ln1_g — the LN1 gain is applied after the pe-add
    # via the ACT copy's scale slot (h = (LN + peb) * g)
    g1 = np.asarray(inputs["ln1_g"], np.float32)
    pe = (_sinusoidal_pe(S, D) + np.asarray(inputs["ln1_b"], np.float32)[None, :]) / g1[None, :]

    indsum = np.zeros((DT * P, 32), np.float32)
    indbc = np.zeros((DT * 32, P), np.float32)
    for t in range(DT):
        for j in range(P):
            h = 2 * t + (1 if j >= 64 else 0)
            indsum[t * P + j, h] = 1.0
            indbc[t * 32 + h, j] = 1.0

    F8NP = ml_dtypes.float8_e4m3
    BFNP = ml_dtypes.bfloat16

    def _pair_pack(w):
        """[K, M] -> [K//2, 2M]: row 128t+p, col j*M+c = w[(2t+j)*128+p, c]."""
        K, M = w.shape
        return np.ascontiguousarray(
            w.reshape(K // 256, 2, P, M).transpose(0, 2, 1, 3).reshape(K // 2, 2 * M))

    wqkv = np.asarray(inputs["w_qkv"], np.float32)
    shared = {
        "wq": _pair_pack(np.ascontiguousarray(wqkv[:, :D])).astype(F8NP),
        "wkv": _pair_pack(np.ascontiguousarray(wqkv[:, D:])).astype(F8NP),
        "wout": _col_block(np.asarray(inputs["w_out"], np.float32)).astype(F8NP),
        "w1": _col_block(np.asarray(inputs["w1"], np.float32)).astype(F8NP),
        "w2": _col_block(np.asarray(inputs["w2"], np.float32)).astype(F8NP),
        "w3": _col_block(np.asarray(inputs["w3"], np.float32)).astype(F8NP),
        "lng1": _vec_tiles(inputs["ln1_g"], DT),
        "lng2": _vec_tiles(inputs["ln2_g"], DT),
        "lnb2": _vec_tiles(inputs["ln2_b"], DT),
        "bout": _vec_tiles(inputs["b_out"], DT),
        "b1": _vec_tiles(inputs["b1"], DDT),
        "b2": _vec_tiles(inputs["b2"], DDT),
        "b3": _vec_tiles(inputs["b3"], DT),
        "ones": np.ones((P, P), np.float32),
        "indsum": indsum.astype(BFNP),
        "indbc": indbc.astype(BFNP),
    }
    in_maps = []
    for c in range(NCORES):
        b, hhalf = divmod(c, NCORES // B)
        s0 = hhalf * T
        m = dict(shared)
        m["xT"] = np.ascontiguousarray(x[b, s0:s0 + T, :].T)
        m["peb"] = np.ascontiguousarray(pe[s0:s0 + T, :].T).astype(BFNP)
        in_maps.append(m)
    return in_maps


def gather(results, S):
    T = B * S // NCORES
    full = np.empty((B, S, D), np.float32)
    for c in range(NCORES):
        b, hhalf = divmod(c, NCORES // B)
        s0 = hhalf * T
        full[b, s0:s0 + T, :] = results[c]["out"].T
    return full


_GRAPH_CACHE = {}


def _get_graph(S):
    T = B * S // NCORES
    if T not in _GRAPH_CACHE:
        _GRAPH_CACHE[T] = build_graph(T)
    return _GRAPH_CACHE[T]


def run(inputs, S, **kw):
    nc = _get_graph(S)
    in_maps = make_in_maps(inputs, S)
    res = run_bass_kernel_spmd(nc, in_maps, core_ids=list(range(NCORES)), **kw)
    return gather(res.results, S), res


def kernel(**inputs):
    out, _ = run(inputs, S_FULL)
    return out
